# revision 1
# baseline (speedup 1.0000x reference)
"""Trainium2 Bass kernel for a dense transformer block (attention + MLP).

Strategy: data-parallel over batch across 8 NeuronCores (48 batches each).
Per core, batches are processed in groups of 4 (512 tokens) so every dense
matmul has a 512-wide moving operand. Activations live transposed in SBUF
([feature, token]) so DRAM-layout weights serve directly as the stationary
matmul operand. Matmul operands are fp16 (full PE rate, 10-bit mantissa);
accumulation is fp32 in PSUM.

Attention is computed k-major to avoid PE transposes entirely: scores are
built as S^T = K_h^T-stationary x Q_h-moving giving [t, s] tiles, exp is
taken UNnormalized into bfloat16 (fp32-range exponent, so no max-subtraction
is needed), the softmax denominators come from a ones-stationary matmul that
broadcasts column sums across all partitions, and normalization happens for
free in the PSUM->SBUF copy after the attnV matmul (scalar_tensor_tensor
multiply by the fast-reciprocal of the sums). The per-batch dependency chain
is scores -> exp -> attnV with everything else off the critical path.

Groups are software-pipelined: group i's attention is followed by group
i-1's MLP1 so the out-projection of group i never waits on the softmax
chain, and MLP2 of group i-1 closes the group.
"""

from contextlib import ExitStack

import numpy as np

B, S, E, H, D, F = 384, 128, 512, 4, 128, 2048
NCORES = 8
BL = B // NCORES  # 48 batches per core
GB = 4  # batches per group
NTOK = GB * S  # 512 tokens per group
KE = E // 128  # 4
KF = F // 128  # 16

MMDT_NP = np.float16  # matmul operand dtype (fp16: full PE rate, 10-bit mantissa)

_cache: dict = {}


# --------------------------------------------------------------------------
# Workaround: the walrus build in this container accepts at most ONE
# sync-wait command per instruction, while Tile emits several. Hoist every
# extra wait onto its own preceding same-engine InstNoOp (engine queues are
# FIFO, so this is semantically identical).
def _fix_multiwaits(nc):
    import concourse.mybir as mybir

    n = 0
    for fn in nc.m.functions:
        for bb in fn.blocks:
            out = []
            changed = False
            for inst in bb.instructions:
                si = inst.sync_info
                if si is not None and len(si.on_wait) > 1:
                    waits = list(si.on_wait)
                    for w in waits[:-1]:
                        n += 1
                        out.append(
                            mybir.InstNoOp(
                                name=f"I-mwfix-{n}",
                                engine=inst.engine,
                                bass_nofuse=True,
                                sync_info=mybir.SyncInfo(on_wait=[w], on_update=[]),
                            )
                        )
                    inst.sync_info = mybir.SyncInfo(
                        on_wait=[waits[-1]], on_update=list(si.on_update)
                    )
                    changed = True
                out.append(inst)
            if changed:
                bb.instructions = out
    return n


def _build(ng, variant="full", repeat=1, cfg=None, mwfix=True):
    """Build the per-core Bass program processing ng groups of 4 batches."""
    import concourse.bass as bass
    import concourse.mybir as mybir
    import concourse.tile as tile

    cfg = {
        **dict(
            big=5, sz=2, atp=1,
            xtp=3, qkp=2, vp=2, pp=4, rzp=2, atsb=2, tmpp=3, xmp=2, hp=2, yp=3,
            nk8=2,  # leading k-tiles of MLP1 done in fp8 DoubleRow (0|2|4)
            nf8=4,  # leading f-tiles of MLP2 done in fp8 DoubleRow (even, 0..16)
        ),
        **(cfg or {}),
    }
    nk8 = cfg["nk8"]
    nf8 = cfg["nf8"]
    gb = GB
    ntok = NTOK
    f32 = mybir.dt.float32
    f16 = mybir.dt.float16
    b16 = mybir.dt.bfloat16
    AF = mybir.ActivationFunctionType
    ALU = mybir.AluOpType
    ts = bass.ts

    ntok_total = BL * S

    nc = bass.Bass("TRN2", target_bir_lowering=False, debug=False)

    xt = nc.dram_tensor("xt", [E, ntok_total], f16, kind="ExternalInput")
    wq_d = nc.dram_tensor("wq", [E, E], f16, kind="ExternalInput")
    wk_d = nc.dram_tensor("wk", [E, E], f16, kind="ExternalInput")
    wv_d = nc.dram_tensor("wv", [E, E], f16, kind="ExternalInput")
    wo_d = nc.dram_tensor("wo", [E, E], f16, kind="ExternalInput")
    w1_d = nc.dram_tensor("w1", [E, F], f16, kind="ExternalInput")
    w1q_d = nc.dram_tensor("w1q", [128, KE, F], mybir.dt.float8e4, kind="ExternalInput")
    w2_d = nc.dram_tensor("w2", [F, E], f16, kind="ExternalInput")
    w2q_d = nc.dram_tensor("w2q", [128, KF, E], mybir.dt.float8e4, kind="ExternalInput")
    bias_d = nc.dram_tensor("bias", [128, 32], f32, kind="ExternalInput")
    yt = nc.dram_tensor("yt", [E, ntok_total], f16, kind="ExternalOutput")

    with tile.TileContext(nc) as tc, ExitStack() as ctx:
        singles = ctx.enter_context(tc.tile_pool(name="singles", bufs=1))

        xtp = ctx.enter_context(tc.tile_pool(name="xtp", bufs=cfg["xtp"]))

        def load_weight(name, dram, n_k, width, engine, split=False):
            t = singles.tile([128, n_k, width], f16, tag=f"w_{name}", name=f"w_{name}")
            if split:
                # per-k-tile DMAs: the first matmul of a chain only needs
                # k-tile 0, so compute starts before the full tile lands
                for k in range(n_k):
                    engine.dma_start(
                        out=t[:, k, :], in_=dram[k * 128 : (k + 1) * 128, :]
                    )
            else:
                engine.dma_start(
                    out=t, in_=dram[:, :].rearrange("(k p) w -> p k w", p=128)
                )
            return [t[:, k, :] for k in range(n_k)]

        # group 0's x load is emitted by the pipeline BEFORE these weight
        # loads land on the rings, so first matmuls start early.
        xt_tiles = {}

        def emit_load(i):
            g = i % ng
            c0 = g * ntok
            xt_t = xtp.tile([128, KE, ntok], f16, tag="xt", name="xt_t")
            for k in range(KE):
                nc.sync.dma_start(
                    out=xt_t[:, k, :],
                    in_=xt[k * 128 : (k + 1) * 128, c0 : c0 + ntok],
                )
            xt_tiles[i] = [xt_t[:, k, :] for k in range(KE)]

        emit_load(0)

        # q/k weights ride the sync ring right behind group 0's x; the rest
        # ride the (otherwise idle at prologue) vector/scalar rings.
        bias_sb = singles.tile([128, 32], f32, tag="b_all", name="b_all")
        nc.gpsimd.dma_start(out=bias_sb, in_=bias_d[:, :])

        # q/k weights ride the gpsimd SWDGE ring, in parallel with group 0's
        # x on the sync ring, so the first matmul starts ~2us earlier
        wq_sb = load_weight("wq", wq_d, KE, E, nc.gpsimd, split=True)
        wk_sb = load_weight("wk", wk_d, KE, E, nc.gpsimd, split=True)
        wv_sb = load_weight("wv", wv_d, KE, E, nc.scalar)
        wo_sb = load_weight("wo", wo_d, KE, E, nc.scalar)
        w1_sb = load_weight("w1", w1_d, KE, F, nc.scalar)
        w2_sb = load_weight("w2", w2_d, KF, E, nc.scalar)
        w1q_sb = singles.tile([128, KE, F], mybir.dt.float8e4, tag="w_w1q", name="w_w1q")
        if nk8:
            nc.scalar.dma_start(out=w1q_sb, in_=w1q_d[:, :, :])
        w2q_sb = singles.tile([128, KF, E], mybir.dt.float8e4, tag="w_w2q", name="w_w2q")
        if nf8:
            nc.scalar.dma_start(out=w2q_sb, in_=w2q_d[:, :, :])

        bq_sb = bias_sb[:, 0:KE]
        bk_sb = bias_sb[:, KE : 2 * KE]
        bo_sb = bias_sb[:, 2 * KE : 3 * KE]
        b1_sb = bias_sb[:, 12 : 12 + KF]
        b2_sb = bias_sb[:, 28 : 28 + KE]

        ones = singles.tile([128, 128], b16, tag="ones")
        nc.gpsimd.memset(ones, 1.0)

        qkp = ctx.enter_context(tc.tile_pool(name="qkp", bufs=cfg["qkp"]))
        vp = ctx.enter_context(tc.tile_pool(name="vp", bufs=cfg["vp"]))
        pp = ctx.enter_context(tc.tile_pool(name="pp", bufs=cfg["pp"]))
        rzp = ctx.enter_context(tc.tile_pool(name="rzp", bufs=cfg["rzp"]))
        atp = ctx.enter_context(tc.tile_pool(name="atp", bufs=cfg["atsb"]))
        tmpp = ctx.enter_context(tc.tile_pool(name="tmpp", bufs=cfg["tmpp"]))
        xmp = ctx.enter_context(tc.tile_pool(name="xmp", bufs=cfg["xmp"]))
        hp = ctx.enter_context(tc.tile_pool(name="hp", bufs=cfg["hp"]))
        yp = ctx.enter_context(tc.tile_pool(name="yp", bufs=cfg["yp"]))

        ps_big = ctx.enter_context(tc.tile_pool(name="ps_big", bufs=cfg["big"], space="PSUM"))
        ps_sz = ctx.enter_context(tc.tile_pool(name="ps_sz", bufs=cfg["sz"], space="PSUM"))
        ps_atp = ctx.enter_context(tc.tile_pool(name="ps_atp", bufs=cfg["atp"], space="PSUM"))

        n_iters = ng * repeat
        st = {}  # per-iteration state

        def emit_qk(i):
            xt_sb = xt_tiles[i]
            s = st[i] = {}
            q_sb, k_sb = [], []
            for which, w_sb, b_sb, dst in (
                ("q", wq_sb, bq_sb, q_sb),
                ("k", wk_sb, bk_sb, k_sb),
            ):
                for h in range(H):
                    ps = ps_big.tile([128, ntok], f32, tag="big", name="qk_ps")
                    for k in range(KE):
                        nc.tensor.matmul(
                            ps,
                            w_sb[k][:, ts(h, 128)],
                            xt_sb[k],
                            start=(k == 0),
                            stop=(k == KE - 1),
                        )
                    t = qkp.tile([128, ntok], f16, tag=f"{which}{h}", name=f"{which}{h}")
                    nc.vector.tensor_scalar_add(t, ps, b_sb[:, h : h + 1])
                    dst.append(t)
            s["q"], s["k"] = q_sb, k_sb

        def emit_v(i):
            s = st[i]
            xt_sb = xt_tiles[i]
            v_sb = []
            for bi in range(gb):
                ps = ps_big.tile([128, E], f32, tag="big", name="v_ps")
                for k in range(KE):
                    nc.tensor.matmul(
                        ps,
                        xt_sb[k][:, ts(bi, 128)],
                        wv_sb[k],
                        start=(k == 0),
                        stop=(k == KE - 1),
                    )
                t = vp.tile([128, E], b16, tag=f"v{bi}", name=f"v{bi}")
                # vector engine (not scalar) so the scalar queue reaches exp()
                # with no backlog — exp is on the attention critical path
                nc.vector.tensor_copy(t, ps)
                v_sb.append(t)
            s["v"] = v_sb
            at_t = atp.tile([128, H, ntok], f16, tag="at", name="at_t")
            s["at_t"] = at_t
            s["at"] = [at_t[:, h, :] for h in range(H)]

        def emit_scores(i, bi):
            # S^T[t, (h,s)] for batch bi, then unnormalized exp in bfloat16
            s = st[i]
            s_ps = ps_sz.tile([128, H * 128], f32, tag="sz", name="s_ps")
            for h in range(H):
                nc.tensor.matmul(
                    s_ps[:, ts(h, 128)],
                    s["k"][h][:, ts(bi, 128)],
                    s["q"][h][:, ts(bi, 128)],
                )
            p_sb = pp.tile([128, H * 128], b16, tag="p", name="p_sb")
            nc.scalar.activation(out=p_sb, in_=s_ps, func=AF.Exp)
            s[f"p{bi}"] = p_sb

        def emit_attn_out(i, bi):
            s = st[i]
            p_sb = s.pop(f"p{bi}")
            # column sums of exp, broadcast to all partitions by the
            # ones-stationary matmul
            z_ps = ps_sz.tile([128, H * 128], f32, tag="sz", name="z_ps")
            nc.tensor.matmul(z_ps, ones, p_sb)
            rz_sb = rzp.tile([128, H * 128], f32, tag="rz", name="rz_sb")
            nc.vector.reciprocal(rz_sb, z_ps)
            at_ps = ps_atp.tile([128, H, 128], f32, tag="atp", name="at_ps")
            for h in range(H):
                nc.tensor.matmul(
                    at_ps[:, h, :], s["v"][bi][:, ts(h, 128)], p_sb[:, ts(h, 128)]
                )
            # normalize while copying out of PSUM: at = at_ps * (1/z)
            nc.vector.tensor_mul(
                s["at_t"][:, :, ts(bi, 128)],
                at_ps,
                rz_sb.rearrange("p (h s) -> p h s", h=H),
            )

        def emit_outproj(i):
            s = st[i]
            xt_sb = xt_tiles.pop(i)
            xm_sb = []
            for m in range(KE):
                ps = ps_big.tile([128, ntok], f32, tag="big", name="o_ps")
                for k in range(KE):
                    nc.tensor.matmul(
                        ps,
                        wo_sb[k][:, ts(m, 128)],
                        s["at"][k],
                        start=(k == 0),
                        stop=(k == KE - 1),
                    )
                tmp = tmpp.tile([128, ntok], f16, tag="tmp", name="tmp")
                nc.scalar.activation(
                    out=tmp, in_=ps, func=AF.Identity, bias=bo_sb[:, m : m + 1]
                )
                xm = xmp.tile([128, ntok], f16, tag=f"xm{m}", name=f"xm{m}")
                nc.gpsimd.tensor_add(xm, tmp, xt_sb[m])
                xm_sb.append(xm)
            s["xm"] = xm_sb
            xm8 = []
            for j in range(nk8 // 2):
                t8 = xmp.tile([128, 2, ntok], mybir.dt.float8e4, tag=f"xm8_{j}", name=f"xm8_{j}")
                for r in range(2):
                    nc.gpsimd.tensor_copy(t8[:, r, :], xm_sb[2 * j + r])
                xm8.append(t8)
            s["xm8"] = xm8

        def emit_mlp1_chunk(i, fs):
            s = st[i]
            h_sb = s.setdefault("h", [])
            for f in fs:
                ps = ps_big.tile([128, ntok], f32, tag="big", name="h_ps")
                for j in range(nk8 // 2):
                    nc.tensor.matmul(
                        ps,
                        w1q_sb[:, 2 * j : 2 * j + 2, ts(f, 128)],
                        s["xm8"][j],
                        start=(j == 0),
                        stop=(nk8 == KE and j == nk8 // 2 - 1),
                        perf_mode=mybir.MatmulPerfMode.DoubleRow,
                    )
                for k in range(nk8, KE):
                    nc.tensor.matmul(
                        ps,
                        w1_sb[k][:, ts(f, 128)],
                        s["xm"][k],
                        start=(k == 0),
                        stop=(k == KE - 1),
                    )
                if f < nf8:
                    # fp8 pair tile feeding MLP2's DoubleRow chain directly
                    j = f // 2
                    if f % 2 == 0:
                        t8 = hp.tile(
                            [128, 2, ntok], mybir.dt.float8e4,
                            tag=f"h8_{j}", name=f"h8_{j}",
                        )
                        s.setdefault("h8", []).append(t8)
                    out_ap = s["h8"][j][:, f % 2, :]
                    t = None
                else:
                    t = hp.tile([128, ntok], f16, tag=f"h{f}", name=f"h{f}")
                    out_ap = t
                if f % 2 == 0:
                    nc.scalar.activation(
                        out=out_ap, in_=ps, func=AF.Relu, bias=b1_sb[:, f : f + 1]
                    )
                else:
                    nc.vector.tensor_scalar(
                        out_ap, ps, b1_sb[:, f : f + 1], 0.0,
                        op0=ALU.add, op1=ALU.max,
                    )
                h_sb.append(t)

        def emit_mlp2_store(i, nq=2):
            s = st[i]
            g = i % ng
            c0 = g * ntok
            h_sb = s["h"]
            yf = yp.tile([128, KE, ntok], f16, tag="yf", name="yf")
            for m in range(KE):
                ps = ps_big.tile([128, ntok], f32, tag="big", name="acc_ps")
                for j in range(nf8 // 2):
                    nc.tensor.matmul(
                        ps,
                        w2q_sb[:, 2 * j : 2 * j + 2, ts(m, 128)],
                        s["h8"][j],
                        start=(j == 0),
                        stop=(nf8 == KF and j == nf8 // 2 - 1),
                        perf_mode=mybir.MatmulPerfMode.DoubleRow,
                    )
                for f in range(nf8, KF):
                    nc.tensor.matmul(
                        ps,
                        w2_sb[f][:, ts(m, 128)],
                        h_sb[f],
                        start=(f == 0),
                        stop=(f == KF - 1),
                    )
                tmpf = tmpp.tile([128, ntok], f32, tag="tmpf", name="tmpf")
                # half-width copy/add/store chain: the store of one half
                # overlaps the residual add of the other, shortening the
                # post-PE drain at the end of the program
                for half in range(nq):
                    hs = slice(half * (ntok // nq), (half + 1) * (ntok // nq))
                    nc.scalar.activation(
                        out=tmpf[:, hs], in_=ps[:, hs], func=AF.Identity,
                        scale=1.0 / 256.0, bias=b2_sb[:, m : m + 1],
                    )
                    nc.gpsimd.tensor_add(yf[:, m, hs], tmpf[:, hs], s["xm"][m][:, hs])
                    nc.scalar.dma_start(
                        out=yt[m * 128 : (m + 1) * 128, c0 + half * (ntok // nq) : c0 + (half + 1) * (ntok // nq)],
                        in_=yf[:, m, hs],
                    )
            del st[i]

        # q/k/v of group i+1 are hoisted into group i's slot: they are pure PE
        # work with no softmax dependencies, so they cover the DVE-bound
        # normalize chain ahead of oproj even on the first groups (ramp).
        if n_iters > 1:
            emit_load(1)
        emit_qk(0)
        emit_v(0)
        for i in range(n_iters):
            if i + 2 < n_iters:
                emit_load(i + 2)
            for bi in range(gb):
                emit_scores(i, bi)
            # next group's projections sit between scores and attn-outs so
            # their DVE bias-adds run ahead of the recip/normalize backlog
            # (the v chains' PSUM ring waits on those bias-adds)
            if i + 1 < n_iters:
                emit_qk(i + 1)
                emit_v(i + 1)
            for bi in range(gb):
                emit_attn_out(i, bi)
            if i >= 1:
                emit_mlp1_chunk(i - 1, range(KF))
            emit_outproj(i)
            if i >= 1:
                emit_mlp2_store(i - 1)
        emit_mlp1_chunk(n_iters - 1, range(KF))
        emit_mlp2_store(n_iters - 1)

    if mwfix:
        _fix_multiwaits(nc)
    return nc


def _get_program(ng, variant="full", repeat=1, cfg=None):
    key = ("nc", ng, variant, repeat, tuple(sorted((cfg or {}).items())))
    if key not in _cache:
        _cache[key] = _build(ng, variant, repeat, cfg)
    return _cache[key]


def make_in_maps(x, wq, bq, wk, bk, wv, bv, wo, bo, w1, b1, w2, b2):
    x = np.asarray(x, np.float32)
    to_bf = lambda a: np.ascontiguousarray(np.asarray(a, np.float32).astype(MMDT_NP))

    # host-side prep: shard + transpose + cast
    ntok_total = BL * S
    x_sh = x.reshape(NCORES, ntok_total, E)
    xts = [np.ascontiguousarray(x_sh[c].T).astype(MMDT_NP) for c in range(NCORES)]

    # MLP scale convention: w1 (and b1) carry x16 so the fp8 copy of w1 sits
    # in e4m3's normal range; h tiles then hold 16*h and w2 carries /16.
    # Both scalings are exact powers of two in fp16.
    import ml_dtypes

    w1_s = np.asarray(w1, np.float32) * 16.0
    w2_s = np.asarray(w2, np.float32) * 16.0
    wq_b, wk_b, wv_b, wo_b, w1_b, w2_b = map(
        to_bf, (wq, wk, wv, wo, w1_s, w2_s)
    )
    w1q = np.ascontiguousarray(
        w1_s.reshape(KE, 128, F).transpose(1, 0, 2)
    ).astype(ml_dtypes.float8_e4m3)
    w2q = np.ascontiguousarray(
        w2_s.reshape(KF, 128, E).transpose(1, 0, 2)
    ).astype(ml_dtypes.float8_e4m3)

    resh = lambda b, nk: np.asarray(b, np.float32).reshape(nk, 128).T
    # bv is folded into the output-projection bias: P rows sum to 1, so
    # attn@wo + bo == (P@v_nobias)@wo + (bv@wo + bo).
    bo_eff = (
        np.asarray(bv, np.float64) @ np.asarray(wo, np.float64)
        + np.asarray(bo, np.float64)
    ).astype(np.float32)
    bias_pack = np.zeros((128, 32), np.float32)
    bias_pack[:, 0:KE] = resh(bq, KE)
    bias_pack[:, KE : 2 * KE] = resh(bk, KE)
    bias_pack[:, 2 * KE : 3 * KE] = resh(bo_eff, KE)
    bias_pack[:, 12 : 12 + KF] = resh(np.asarray(b1, np.float32) * 16.0, KF)
    bias_pack[:, 28 : 28 + KE] = resh(b2, KE)

    in_maps = []
    for c in range(NCORES):
        in_maps.append(
            {
                "xt": xts[c],
                "wq": wq_b,
                "wk": wk_b,
                "wv": wv_b,
                "wo": wo_b,
                "w1": w1_b,
                "w1q": w1q,
                "w2": w2_b,
                "w2q": w2q,
                "bias": bias_pack,
            }
        )
    return in_maps


def kernel(
    x, wq, bq, wk, bk, wv, bv, wo, bo, w1, b1, w2, b2, _ng=BL // GB
):
    import os

    from concourse.bass_utils import run_bass_kernel_spmd

    # The NTFF trace hook module does not exist in this container; make sure
    # run_bass_kernel_spmd never takes the trace branch even if BASS_TRACE
    # is set in the environment.
    os.environ["BASS_NEVER_TRACE"] = "1"

    in_maps = make_in_maps(x, wq, bq, wk, bk, wv, bv, wo, bo, w1, b1, w2, b2)
    ntok_total = BL * S
    nc = _get_program(_ng)

    res = run_bass_kernel_spmd(nc, in_maps, core_ids=list(range(NCORES)))
    _cache["last_result"] = res

    out = np.empty((NCORES, ntok_total, E), np.float32)
    for c in range(NCORES):
        out[c] = res.results[c]["yt"].T.astype(np.float32)
    return out.reshape(B, S, E)



# revision 42
# speedup vs baseline: 1.2048x; 1.2048x over previous
"""Trainium2 Bass kernel for a dense transformer block (attention + MLP).

Strategy: data-parallel over batch across 8 NeuronCores (48 batches each).
Per core, batches are processed in groups of 4 (512 tokens) so every dense
matmul has a 512-wide moving operand. Activations live transposed in SBUF
([feature, token]) so DRAM-layout weights serve directly as the stationary
matmul operand. Attention matmul operands are fp16 (full PE rate, 10-bit
mantissa); accumulation is fp32 in PSUM.

Attention is computed k-major to avoid PE transposes entirely: scores are
built as S^T = K_h^T-stationary x Q_h-moving giving [t, s] tiles, exp is
taken UNnormalized into bfloat16 (fp32-range exponent, so no max-subtraction
is needed), the softmax denominators come from a ones-stationary matmul that
broadcasts column sums across all partitions, and normalization happens for
free in the PSUM->SBUF copy after the attnV matmul.

The MLP runs entirely in fp8 DoubleRow at 2x PE rate with near-fp16
accuracy via an exact-activation decomposition: each DR pass's two virtual
rows carry (q8(a), q8(a - q8(a))) for ONE 128-deep reduction tile against
duplicated fp8 weights, so the activation quantization error cancels to
second order and only the weight rounding error remains. That weight error
is then minimized offline (host-side) by adaptive rounding (coordinate
descent on the fp8 rounding choices against the empirical activation Gram
matrix, fit on a proxy pipeline driven by x).

Groups are software-pipelined: group i's attention is followed by group
i-1's MLP1 so the out-projection of group i never waits on the softmax
chain, and MLP2 of group i-1 closes the group.
"""

from contextlib import ExitStack

import numpy as np

B, S, E, H, D, F = 384, 128, 512, 4, 128, 2048
NCORES = 8
BL = B // NCORES  # 48 batches per core
GB = 4  # batches per group
NTOK = GB * S  # 512 tokens per group
KE = E // 128  # 4
KF = F // 128  # 16

MMDT_NP = np.float16  # attention matmul operand dtype

_cache: dict = {}


# --------------------------------------------------------------------------
# Workaround: the walrus build in this container accepts at most ONE
# sync-wait command per instruction, while Tile emits several. Hoist every
# extra wait onto its own preceding same-engine InstNoOp (engine queues are
# FIFO, so this is semantically identical).
def _fix_multiwaits(nc):
    import concourse.mybir as mybir

    n = 0
    for fn in nc.m.functions:
        for bb in fn.blocks:
            out = []
            changed = False
            for inst in bb.instructions:
                si = inst.sync_info
                if si is not None and len(si.on_wait) > 1:
                    waits = list(si.on_wait)
                    for w in waits[:-1]:
                        n += 1
                        out.append(
                            mybir.InstNoOp(
                                name=f"I-mwfix-{n}",
                                engine=inst.engine,
                                bass_nofuse=True,
                                sync_info=mybir.SyncInfo(on_wait=[w], on_update=[]),
                            )
                        )
                    inst.sync_info = mybir.SyncInfo(
                        on_wait=[waits[-1]], on_update=list(si.on_update)
                    )
                    changed = True
                out.append(inst)
            if changed:
                bb.instructions = out
    return n


def _build(ng, variant="full", repeat=1, cfg=None, mwfix=True):
    """Build the per-core Bass program processing ng groups of 4 batches."""
    import concourse.bass as bass
    import concourse.mybir as mybir
    import concourse.tile as tile

    cfg = {
        **dict(
            big=5, sz=2, atp=1,
            xtp=3, qkp=2, vp=2, pp=4, rzp=2, atsb=2, tmpp=3, xmp=2, hp=2, yp=3,
            htp=5,
            mlp1_f16=0,  # leading k-tiles of MLP1 kept in fp16 (error knob)
            mlp2_f16=0,  # leading f-tiles of MLP2 kept in fp16 (error knob)
            m1p2=2,  # trailing MLP2 f-pairs as plain fp8 pairs (2 tiles/DR)
            zb=True,  # zero-bias fast path for the h fp8 pair (b1 == 0)
        ),
        **(cfg or {}),
    }
    n1f = cfg["mlp1_f16"]
    n2f = cfg["mlp2_f16"]
    m1p2 = cfg["m1p2"]
    f_m1_start = KF - 2 * m1p2  # f-tiles >= this are plain-fp8 paired
    zb = cfg["zb"]
    gb = GB
    ntok = NTOK
    f32 = mybir.dt.float32
    f16 = mybir.dt.float16
    b16 = mybir.dt.bfloat16
    fp8 = mybir.dt.float8e4
    AF = mybir.ActivationFunctionType
    ALU = mybir.AluOpType
    ts = bass.ts

    ntok_total = BL * S

    nc = bass.Bass("TRN2", target_bir_lowering=False, debug=False)

    xt = nc.dram_tensor("xt", [E, ntok_total], f16, kind="ExternalInput")
    wq_d = nc.dram_tensor("wq", [E, E], f16, kind="ExternalInput")
    wk_d = nc.dram_tensor("wk", [E, E], f16, kind="ExternalInput")
    wv_d = nc.dram_tensor("wv", [E, E], f16, kind="ExternalInput")
    wo_d = nc.dram_tensor("wo", [E, E], f16, kind="ExternalInput")
    w1p_d = nc.dram_tensor("w1p", [128, KE, 2, F], fp8, kind="ExternalInput")
    w2p_d = nc.dram_tensor("w2p", [128, KF, 2, E], fp8, kind="ExternalInput")
    w1_d = nc.dram_tensor("w1", [E, F], f16, kind="ExternalInput")
    w2_d = nc.dram_tensor("w2", [F, E], f16, kind="ExternalInput")
    bias_d = nc.dram_tensor("bias", [128, 32], f32, kind="ExternalInput")
    yt = nc.dram_tensor("yt", [E, ntok_total], f16, kind="ExternalOutput")

    with tile.TileContext(nc) as tc, ExitStack() as ctx:
        singles = ctx.enter_context(tc.tile_pool(name="singles", bufs=1))

        xtp = ctx.enter_context(tc.tile_pool(name="xtp", bufs=cfg["xtp"]))

        def load_weight(name, dram, n_k, width, engine, split=False):
            t = singles.tile([128, n_k, width], f16, tag=f"w_{name}", name=f"w_{name}")
            if split:
                # per-k-tile DMAs: the first matmul of a chain only needs
                # k-tile 0, so compute starts before the full tile lands
                for k in range(n_k):
                    engine.dma_start(
                        out=t[:, k, :], in_=dram[k * 128 : (k + 1) * 128, :]
                    )
            else:
                engine.dma_start(
                    out=t, in_=dram[:, :].rearrange("(k p) w -> p k w", p=128)
                )
            return [t[:, k, :] for k in range(n_k)]

        # group 0's x load is emitted by the pipeline BEFORE these weight
        # loads land on the rings, so first matmuls start early.
        xt_tiles = {}

        def emit_load(i):
            g = i % ng
            c0 = g * ntok
            xt_t = xtp.tile([128, KE, ntok], f16, tag="xt", name="xt_t")
            for k in range(KE):
                nc.sync.dma_start(
                    out=xt_t[:, k, :],
                    in_=xt[k * 128 : (k + 1) * 128, c0 : c0 + ntok],
                )
            xt_tiles[i] = [xt_t[:, k, :] for k in range(KE)]

        emit_load(0)

        # q/k weights ride the gpsimd SWDGE ring, in parallel with group 0's
        # x on the sync ring, so the first matmul starts ~2us earlier.
        # NOTHING may ride the scalar ring early: ACT compute ops (q-copies,
        # exp) queue behind same-engine DMAs and would hold PSUM banks hostage
        wq_sb = load_weight("wq", wq_d, KE, E, nc.gpsimd, split=True)
        wk_sb = load_weight("wk", wk_d, KE, E, nc.gpsimd, split=True)
        # the bias is tiny: its 500ns issue on the scalar ring finishes long
        # before ACT's first compute op, unlike the big weight DMAs
        bias_sb = singles.tile([128, 32], f32, tag="b_all", name="b_all")
        nc.scalar.dma_start(out=bias_sb, in_=bias_d[:, :])
        wv_sb = load_weight("wv", wv_d, KE, E, nc.gpsimd)
        wo_sb = load_weight("wo", wo_d, KE, E, nc.gpsimd)
        # the big fp8 MLP weights ride the sync ring behind the first two
        # x-group loads; they are not needed until MLP1 of group 0 (~25us)
        w1p_sb = singles.tile([128, KE, 2, F], fp8, tag="w_w1p", name="w_w1p")
        w2p_sb = singles.tile([128, KF, 2, E], fp8, tag="w_w2p", name="w_w2p")
        nc.sync.dma_start(out=w1p_sb, in_=w1p_d[:, :, :, :])
        nc.sync.dma_start(out=w2p_sb, in_=w2p_d[:, :, :, :])
        if n1f:
            w1_sb = load_weight("w1", w1_d, KE, F, nc.sync)
        if n2f:
            w2_sb = load_weight("w2", w2_d, KF, E, nc.sync)

        bq_sb = bias_sb[:, 0:KE]
        bk_sb = bias_sb[:, KE : 2 * KE]
        bo_sb = bias_sb[:, 2 * KE : 3 * KE]
        b1_sb = bias_sb[:, 12 : 12 + KF]
        b2_sb = bias_sb[:, 28 : 28 + KE]

        ones = singles.tile([128, 128], b16, tag="ones")
        nc.gpsimd.memset(ones, 1.0)

        qkp = ctx.enter_context(tc.tile_pool(name="qkp", bufs=cfg["qkp"]))
        vp = ctx.enter_context(tc.tile_pool(name="vp", bufs=cfg["vp"]))
        pp = ctx.enter_context(tc.tile_pool(name="pp", bufs=cfg["pp"]))
        rzp = ctx.enter_context(tc.tile_pool(name="rzp", bufs=cfg["rzp"]))
        atp = ctx.enter_context(tc.tile_pool(name="atp", bufs=cfg["atsb"]))
        tmpp = ctx.enter_context(tc.tile_pool(name="tmpp", bufs=cfg["tmpp"]))
        xmp = ctx.enter_context(tc.tile_pool(name="xmp", bufs=cfg["xmp"]))
        hp = ctx.enter_context(tc.tile_pool(name="hp", bufs=cfg["hp"]))
        htp = ctx.enter_context(tc.tile_pool(name="htp", bufs=cfg["htp"]))
        yp = ctx.enter_context(tc.tile_pool(name="yp", bufs=cfg["yp"]))

        ps_big = ctx.enter_context(tc.tile_pool(name="ps_big", bufs=cfg["big"], space="PSUM"))
        ps_sz = ctx.enter_context(tc.tile_pool(name="ps_sz", bufs=cfg["sz"], space="PSUM"))
        ps_atp = ctx.enter_context(tc.tile_pool(name="ps_atp", bufs=cfg["atp"], space="PSUM"))

        n_iters = ng * repeat
        st = {}  # per-iteration state

        def emit_qk(i):
            xt_sb = xt_tiles[i]
            s = st[i] = {}
            q_sb, k_sb = [], []
            for which, w_sb, b_sb, dst in (
                ("q", wq_sb, bq_sb, q_sb),
                ("k", wk_sb, bk_sb, k_sb),
            ):
                for h in range(H):
                    ps = ps_big.tile([128, ntok], f32, tag="big", name="qk_ps")
                    for k in range(KE):
                        nc.tensor.matmul(
                            ps,
                            w_sb[k][:, ts(h, 128)],
                            xt_sb[k],
                            start=(k == 0),
                            stop=(k == KE - 1),
                        )
                    t = qkp.tile([128, ntok], f16, tag=f"{which}{h}", name=f"{which}{h}")
                    # split PSUM-drain ops between ACT (q) and DVE (k)
                    if which == "q":
                        nc.scalar.activation(
                            out=t, in_=ps, func=AF.Identity, bias=b_sb[:, h : h + 1]
                        )
                    else:
                        nc.vector.tensor_scalar_add(t, ps, b_sb[:, h : h + 1])
                    dst.append(t)
            s["q"], s["k"] = q_sb, k_sb

        def emit_v(i):
            s = st[i]
            xt_sb = xt_tiles[i]
            v_sb = []
            for bi in range(gb):
                ps = ps_big.tile([128, E], f32, tag="big", name="v_ps")
                for k in range(KE):
                    nc.tensor.matmul(
                        ps,
                        xt_sb[k][:, ts(bi, 128)],
                        wv_sb[k],
                        start=(k == 0),
                        stop=(k == KE - 1),
                    )
                t = vp.tile([128, E], b16, tag=f"v{bi}", name=f"v{bi}")
                nc.scalar.activation(out=t, in_=ps, func=AF.Identity)
                v_sb.append(t)
            s["v"] = v_sb
            at_t = atp.tile([128, H, ntok], f16, tag="at", name="at_t")
            s["at_t"] = at_t
            s["at"] = [at_t[:, h, :] for h in range(H)]

        def emit_scores(i, bi):
            # S^T[t, (h,s)] for batch bi, then unnormalized exp in bfloat16
            s = st[i]
            s_ps = ps_sz.tile([128, H * 128], f32, tag="sz", name="s_ps")
            for h in range(H):
                nc.tensor.matmul(
                    s_ps[:, ts(h, 128)],
                    s["k"][h][:, ts(bi, 128)],
                    s["q"][h][:, ts(bi, 128)],
                )
            p_sb = pp.tile([128, H * 128], b16, tag="p", name="p_sb")
            nc.scalar.activation(out=p_sb, in_=s_ps, func=AF.Exp)
            s[f"p{bi}"] = p_sb

        def emit_attn_out(i, bi):
            s = st[i]
            p_sb = s.pop(f"p{bi}")
            # column sums of exp, broadcast to all partitions by the
            # ones-stationary matmul
            z_ps = ps_sz.tile([128, H * 128], f32, tag="sz", name="z_ps")
            nc.tensor.matmul(z_ps, ones, p_sb)
            rz_sb = rzp.tile([128, H * 128], f32, tag="rz", name="rz_sb")
            nc.vector.reciprocal(rz_sb, z_ps)
            at_ps = ps_atp.tile([128, H, 128], f32, tag="atp", name="at_ps")
            for h in range(H):
                nc.tensor.matmul(
                    at_ps[:, h, :], s["v"][bi][:, ts(h, 128)], p_sb[:, ts(h, 128)]
                )
            # normalize while copying out of PSUM: at = at_ps * (1/z)
            nc.vector.tensor_mul(
                s["at_t"][:, :, ts(bi, 128)],
                at_ps,
                rz_sb.rearrange("p (h s) -> p h s", h=H),
            )

        def emit_outproj(i):
            s = st[i]
            xt_sb = xt_tiles.pop(i)
            xm_sb = []
            xmx_sb = []
            for m in range(KE):
                ps = ps_big.tile([128, ntok], f32, tag="big", name="o_ps")
                for k in range(KE):
                    nc.tensor.matmul(
                        ps,
                        wo_sb[k][:, ts(m, 128)],
                        s["at"][k],
                        start=(k == 0),
                        stop=(k == KE - 1),
                    )
                xm = xmp.tile([128, ntok], f16, tag=f"xm{m}", name=f"xm{m}")
                if zb:
                    # bo_eff == 0: fold the residual add into the PSUM drain
                    nc.vector.tensor_add(xm, ps, xt_sb[m])
                else:
                    tmp = tmpp.tile([128, ntok], f16, tag="tmp", name="tmp")
                    nc.scalar.activation(
                        out=tmp, in_=ps, func=AF.Identity, bias=bo_sb[:, m : m + 1]
                    )
                    nc.gpsimd.tensor_add(xm, tmp, xt_sb[m])
                xm_sb.append(xm)
                if m >= n1f:
                    # exact-activation DR pair: plane0 = q8(xm),
                    # plane1 = q8(xm - plane0)
                    xx = xmp.tile(
                        [128, 2, ntok], mybir.dt.float8e4, tag=f"xx{m}", name=f"xx{m}"
                    )
                    nc.gpsimd.tensor_copy(xx[:, 0, :], xm)
                    nc.gpsimd.tensor_sub(xx[:, 1, :], xm, xx[:, 0, :])
                    xmx_sb.append(xx)
                else:
                    xmx_sb.append(None)
            s["xm"] = xm_sb
            s["xmx"] = xmx_sb

        def emit_mlp1_chunk(i, fs, last=False):
            s = st[i]
            h_sb = s.setdefault("hx", [])
            for f in fs:
                ps = ps_big.tile([128, ntok], f32, tag="big", name="h_ps")
                for k in range(n1f):
                    nc.tensor.matmul(
                        ps,
                        w1_sb[k][:, ts(f, 128)],
                        s["xm"][k],
                        start=(k == 0),
                        stop=False,
                    )
                for k in range(n1f, KE):
                    nc.tensor.matmul(
                        ps,
                        w1p_sb[:, k, :, ts(f, 128)],
                        s["xmx"][k],
                        start=(k == 0),
                        stop=(k == KE - 1),
                        perf_mode=mybir.MatmulPerfMode.DoubleRow,
                    )
                if f < n2f:
                    # f16 h tile feeding MLP2's fp16 chain
                    t = hp.tile([128, ntok], f16, tag=f"h{f}", name=f"h{f}")
                    if f % 2 == 0:
                        nc.scalar.activation(
                            out=t, in_=ps, func=AF.Relu, bias=b1_sb[:, f : f + 1]
                        )
                    else:
                        nc.vector.tensor_scalar(
                            t, ps, b1_sb[:, f : f + 1], 0.0, op0=ALU.add, op1=ALU.max
                        )
                    h_sb.append(t)
                else:
                    ht = htp.tile([128, ntok], f16, tag="ht", name="ht")
                    if f % 2 == 0:
                        nc.scalar.activation(
                            out=ht, in_=ps, func=AF.Relu, bias=b1_sb[:, f : f + 1]
                        )
                    else:
                        nc.vector.tensor_scalar(
                            ht, ps, b1_sb[:, f : f + 1], 0.0,
                            op0=ALU.add, op1=ALU.max,
                        )
                    if f >= f_m1_start:
                        # plain fp8 pair: two f-tiles share one DR pass
                        if f % 2 == 0:
                            hx = hp.tile(
                                [128, 2, ntok], mybir.dt.float8e4,
                                tag=f"hx{f}", name=f"hx{f}",
                            )
                            h_sb.append(hx)
                        else:
                            hx = h_sb[f - 1]
                            h_sb.append(None)
                        nc.gpsimd.tensor_copy(hx[:, f % 2, :], ht)
                    else:
                        # exact-activation DR pair: one PSUM drain (relu ->
                        # f16), then SBUF-only fp8 pair production on gpsimd
                        hx = hp.tile(
                            [128, 2, ntok], mybir.dt.float8e4,
                            tag=f"hx{f}", name=f"hx{f}",
                        )
                        # the epilogue has no next-group attention to cover
                        # the pair frontier, so split it across two engines
                        peng = nc.vector if (last and f % 2) else nc.gpsimd
                        peng.tensor_copy(hx[:, 0, :], ht)
                        peng.tensor_sub(hx[:, 1, :], ht, hx[:, 0, :])
                        h_sb.append(hx)

        def emit_relu_halved(ps, out, bias_ap):
            # split the PSUM drain across ACT and DVE so the bank frees in
            # half the time (the drain latency gates the PE chain rotation)
            hm = NTOK // 2
            nc.scalar.activation(
                out=out[:, :hm], in_=ps[:, :hm], func=AF.Relu, bias=bias_ap
            )
            nc.vector.tensor_scalar(
                out[:, hm:], ps[:, hm:], bias_ap, 0.0, op0=ALU.add, op1=ALU.max
            )

        def emit_mlp2_store(i, nq=2):
            s = st[i]
            g = i % ng
            c0 = g * ntok
            h_sb = s["hx"]
            yf = yp.tile([128, KE, ntok], f16, tag="yf", name="yf")
            for m in range(KE):
                ps = ps_big.tile([128, ntok], f32, tag="big", name="acc_ps")
                tmpf = None
                for f in range(n2f):
                    nc.tensor.matmul(
                        ps,
                        w2_sb[f][:, ts(m, 128)],
                        h_sb[f],
                        start=(f == 0),
                        stop=False,
                    )
                for f in range(n2f, KF):
                    if f >= f_m1_start:
                        if f % 2:
                            continue  # odd tile rides its pair's DR pass
                        # plain pair: planes step over adjacent f-tiles
                        stat = w2p_sb[:, f : f + 2, 0, ts(m, 128)]
                    else:
                        stat = w2p_sb[:, f, :, ts(m, 128)]
                    nc.tensor.matmul(
                        ps,
                        stat,
                        h_sb[f],
                        start=(f == 0),
                        stop=(f == KF - 1 or f == f_m1_start + 2 * m1p2 - 2),
                        perf_mode=mybir.MatmulPerfMode.DoubleRow,
                    )
                if not zb:
                    tmpf = tmpp.tile([128, ntok], f32, tag="tmpf", name="tmpf")
                # half-width copy/add/store chain: the store of one half
                # overlaps the residual add of the other, shortening the
                # post-PE drain at the end of the program
                for half in range(nq):
                    hs = slice(half * (ntok // nq), (half + 1) * (ntok // nq))
                    if zb:
                        # b2 == 0: drain + scale + residual in one DVE op
                        nc.vector.scalar_tensor_tensor(
                            yf[:, m, hs], ps[:, hs], 1.0 / 256.0,
                            s["xm"][m][:, hs], op0=ALU.mult, op1=ALU.add,
                        )
                    else:
                        nc.scalar.activation(
                            out=tmpf[:, hs], in_=ps[:, hs], func=AF.Identity,
                            scale=1.0 / 256.0, bias=b2_sb[:, m : m + 1],
                        )
                        nc.gpsimd.tensor_add(
                            yf[:, m, hs], tmpf[:, hs], s["xm"][m][:, hs]
                        )
                    # alternate store rings: a same-engine DMA issue occupies
                    # its queue for ~500ns and would stall compute ops
                    dmae = (nc.scalar, nc.sync, nc.gpsimd, nc.sync)[
                        (2 * m + half) % 4
                    ]
                    dmae.dma_start(
                        out=yt[m * 128 : (m + 1) * 128, c0 + half * (ntok // nq) : c0 + (half + 1) * (ntok // nq)],
                        in_=yf[:, m, hs],
                    )
            del st[i]

        # q/k/v of group i+1 are hoisted into group i's slot: they are pure PE
        # work with no softmax dependencies, so they cover the DVE-bound
        # normalize chain ahead of oproj even on the first groups (ramp).
        if n_iters > 1:
            emit_load(1)
        emit_qk(0)
        emit_v(0)
        for i in range(n_iters):
            if i + 2 < n_iters:
                emit_load(i + 2)
            if i == n_iters - 1 and i >= 1:
                # last slot has no next-group projections to cover the
                # scores->exp->attnv latency chains; interleave the previous
                # group's MLP1 chains instead
                for bi in range(gb):
                    emit_scores(i, bi)
                    emit_mlp1_chunk(i - 1, range(bi * 2, bi * 2 + 2))
                for bi in range(gb):
                    emit_attn_out(i, bi)
                    emit_mlp1_chunk(i - 1, range(8 + bi * 2, 8 + bi * 2 + 2))
                emit_outproj(i)
                emit_mlp2_store(i - 1)
                emit_mlp1_chunk(i, range(KF), last=True)
                emit_mlp2_store(i)
            else:
                for bi in range(gb):
                    emit_scores(i, bi)
                # next group's projections sit between scores and attn-outs so
                # their DVE bias-adds run ahead of the recip/normalize backlog
                # (the v chains' PSUM ring waits on those bias-adds)
                if i + 1 < n_iters:
                    emit_qk(i + 1)
                    emit_v(i + 1)
                for bi in range(gb):
                    emit_attn_out(i, bi)
                if i >= 1:
                    emit_mlp1_chunk(i - 1, range(KF))
                emit_outproj(i)
                if i >= 1:
                    emit_mlp2_store(i - 1)
        if n_iters == 1:
            emit_mlp1_chunk(0, range(KF), last=True)
            emit_mlp2_store(0)

    if mwfix:
        _fix_multiwaits(nc)
    return nc


def _get_program(ng, variant="full", repeat=1, cfg=None):
    key = ("nc", ng, variant, repeat, tuple(sorted((cfg or {}).items())))
    if key not in _cache:
        _cache[key] = _build(ng, variant, repeat, cfg)
    return _cache[key]


# --------------------------------------------------------------------------
# Host-side adaptive rounding of the fp8 MLP weights: minimize
# ||acts @ (w_opt - w)||_F over the per-element choice between the two
# neighboring fp8 grid points, by exact sequential coordinate descent
# (vectorized across output columns).
def _fp8_neighbor_toward(w, wq):
    import ml_dtypes

    b = wq.astype(ml_dtypes.float8_e4m3fn).view(np.uint8)
    sign = (b & 0x80) != 0
    mag = (b & 0x7F).astype(np.int16)
    d = w - wq
    up = d > 0
    newmag = np.where(up ^ sign, mag + 1, mag - 1)
    flip = newmag < 0
    newmag = np.where(flip, 1, newmag)
    newsign = sign ^ flip
    newmag = np.clip(newmag, 0, 0x7E)
    out = newmag.astype(np.uint8) | np.where(newsign, 0x80, 0).astype(np.uint8)
    res = out.view(ml_dtypes.float8_e4m3fn).astype(np.float32)
    return np.where(d == 0, wq, res)


def _q8(a):
    import ml_dtypes

    return np.asarray(a, np.float32).astype(ml_dtypes.float8_e4m3fn).astype(np.float32)


def _adaround(w, acts, n_sweeps=4):
    a = np.ascontiguousarray(acts, np.float32)
    G = (a.T @ a) / len(a)
    wq = _q8(w)
    alt = _fp8_neighbor_toward(w, wq)
    cur = wq.copy()
    K = w.shape[0]
    delta = cur - w
    g = G @ delta
    Gd = G.diagonal()
    for _ in range(n_sweeps):
        nflip = 0
        for i in range(K):
            other = np.where(cur[i] == wq[i], alt[i], wq[i])
            d = other - cur[i]
            gain = 2 * d * g[i] + d * d * Gd[i]
            m = gain < -1e-14
            if m.any():
                du = np.where(m, d, 0.0)
                cur[i] += du
                g += np.outer(G[:, i], du)
                nflip += int(m.sum())
        if nflip == 0:
            break
    return cur


def make_in_maps(x, wq, bq, wk, bk, wv, bv, wo, bo, w1, b1, w2, b2,
                 ada_sweeps=4, ada_sample=3072):
    import ml_dtypes

    x = np.asarray(x, np.float32)
    to_16 = lambda a: np.ascontiguousarray(np.asarray(a, np.float32).astype(MMDT_NP))

    # host-side prep: shard + transpose + cast
    ntok_total = BL * S
    x_sh = x.reshape(NCORES, ntok_total, E)
    xts = [np.ascontiguousarray(x_sh[c].T).astype(MMDT_NP) for c in range(NCORES)]

    # MLP scale convention: w1 (and b1) carry x16 so the fp8 weights sit in
    # e4m3's normal range; h tiles then hold 16*h and w2 carries x16 too, so
    # the MLP2 PSUM result is 256x and one scale of 1/256 restores it.
    # Both scalings are exact powers of two.
    w1_s = np.asarray(w1, np.float32) * 16.0
    w2_s = np.asarray(w2, np.float32) * 16.0
    b1_s = np.asarray(b1, np.float32) * 16.0

    # --- adaptive rounding of the fp8 weights, fit on a sampled-batch
    # attention forward pass so the Gram matrices see the true MLP inputs ---
    q16_ = lambda a: a.astype(np.float16).astype(np.float32)
    nbs = max(1, ada_sample // S)
    xs = x.reshape(B, S, E)[:: max(1, B // nbs)][:nbs]  # [nbs, S, E]
    xs16 = q16_(xs.reshape(nbs * S, E))
    wq16, wk16, wv16, wo16 = (
        q16_(np.asarray(w, np.float32)) for w in (wq, wk, wv, wo)
    )
    bo_eff_f = (
        np.asarray(bv, np.float64) @ np.asarray(wo, np.float64)
        + np.asarray(bo, np.float64)
    ).astype(np.float32)
    qs = q16_(xs16 @ wq16 + np.asarray(bq, np.float32))
    ks = q16_(xs16 @ wk16 + np.asarray(bk, np.float32))
    vs = xs16 @ wv16
    qh = qs.reshape(nbs, S, H, D)
    kh = ks.reshape(nbs, S, H, D)
    vh = vs.reshape(nbs, S, H, D)
    sc = np.einsum("bshd,bthd->bhst", qh, kh, optimize=True)
    sc -= sc.max(axis=-1, keepdims=True)
    p = np.exp(sc, dtype=np.float32)
    p /= p.sum(axis=-1, keepdims=True)
    attn = np.einsum("bhst,bthd->bshd", p, vh, optimize=True)
    at_s = q16_(attn.reshape(nbs * S, E))
    xm_s = q16_(q16_(at_s @ wo16 + bo_eff_f) + xs16)

    x8 = _q8(xm_s)
    axx = x8 + _q8(xm_s - x8)
    if ada_sweeps > 0:
        w1opt = _adaround(w1_s, axx, ada_sweeps)
    else:
        w1opt = _q8(w1_s)
    hprox = np.maximum(axx @ w1opt + b1_s, 0.0)
    h8 = _q8(hprox)
    ahh = h8 + _q8(hprox.astype(np.float16).astype(np.float32) - h8)
    if ada_sweeps > 0:
        w2opt = _adaround(w2_s, ahh, ada_sweeps)
    else:
        w2opt = _q8(w2_s)

    # dual-plane stationary layout [128, K, 2, M]: both planes carry the
    # same (ada-rounded) fp8 weights; the moving pair holds (q8(a), resid)
    w1pairs = np.ascontiguousarray(
        np.broadcast_to(
            w1opt.reshape(KE, 128, 1, F).transpose(1, 0, 2, 3), (128, KE, 2, F)
        )
    ).astype(ml_dtypes.float8_e4m3)
    w2pairs = np.ascontiguousarray(
        np.broadcast_to(
            w2opt.reshape(KF, 128, 1, E).transpose(1, 0, 2, 3), (128, KF, 2, E)
        )
    ).astype(ml_dtypes.float8_e4m3)

    wq_b, wk_b, wv_b, wo_b, w1_b, w2_b = map(
        to_16, (wq, wk, wv, wo, w1_s, w2_s)
    )

    resh = lambda b, nk: np.asarray(b, np.float32).reshape(nk, 128).T
    # bv is folded into the output-projection bias: P rows sum to 1, so
    # attn@wo + bo == (P@v_nobias)@wo + (bv@wo + bo).
    bo_eff = (
        np.asarray(bv, np.float64) @ np.asarray(wo, np.float64)
        + np.asarray(bo, np.float64)
    ).astype(np.float32)
    bias_pack = np.zeros((128, 32), np.float32)
    bias_pack[:, 0:KE] = resh(bq, KE)
    bias_pack[:, KE : 2 * KE] = resh(bk, KE)
    bias_pack[:, 2 * KE : 3 * KE] = resh(bo_eff, KE)
    bias_pack[:, 12 : 12 + KF] = resh(b1_s, KF)
    bias_pack[:, 28 : 28 + KE] = resh(b2, KE)

    in_maps = []
    for c in range(NCORES):
        in_maps.append(
            {
                "xt": xts[c],
                "wq": wq_b,
                "wk": wk_b,
                "wv": wv_b,
                "wo": wo_b,
                "w1p": w1pairs,
                "w2p": w2pairs,
                "w1": w1_b,
                "w2": w2_b,
                "bias": bias_pack,
            }
        )
    return in_maps


def kernel(
    x, wq, bq, wk, bk, wv, bv, wo, bo, w1, b1, w2, b2, _ng=BL // GB, _cfg=None
):
    import os

    from concourse.bass_utils import run_bass_kernel_spmd

    # The NTFF trace hook module does not exist in this container; make sure
    # run_bass_kernel_spmd never takes the trace branch even if BASS_TRACE
    # is set in the environment.
    os.environ["BASS_NEVER_TRACE"] = "1"

    in_maps = make_in_maps(x, wq, bq, wk, bk, wv, bv, wo, bo, w1, b1, w2, b2)
    ntok_total = BL * S
    cfg = dict(_cfg or {})
    # the zb fast paths fold bo_eff (= bv @ wo + bo) and b2 into fused
    # drain+residual ops; they require those biases to be exactly zero
    bo_eff = np.asarray(bv, np.float64) @ np.asarray(wo, np.float64) + np.asarray(
        bo, np.float64
    )
    if not (np.all(bo_eff == 0.0) and np.all(np.asarray(b2) == 0.0)):
        cfg["zb"] = False
    nc = _get_program(_ng, cfg=cfg)

    res = run_bass_kernel_spmd(nc, in_maps, core_ids=list(range(NCORES)))
    _cache["last_result"] = res

    out = np.empty((NCORES, ntok_total, E), np.float32)
    for c in range(NCORES):
        out[c] = res.results[c]["yt"].T.astype(np.float32)
    return out.reshape(B, S, E)


# revision 50
# speedup vs baseline: 1.2063x; 1.0012x over previous
"""Trainium2 Bass kernel for a dense transformer block (attention + MLP).

Strategy: data-parallel over batch across 8 NeuronCores (48 batches each).
Per core, batches are processed in groups of 4 (512 tokens) so every dense
matmul has a 512-wide moving operand. Activations live transposed in SBUF
([feature, token]) so DRAM-layout weights serve directly as the stationary
matmul operand. Attention matmul operands are fp16 (full PE rate, 10-bit
mantissa); accumulation is fp32 in PSUM.

Attention is computed k-major to avoid PE transposes entirely: scores are
built as S^T = K_h^T-stationary x Q_h-moving giving [t, s] tiles, exp is
taken UNnormalized into bfloat16 (fp32-range exponent, so no max-subtraction
is needed), the softmax denominators come from a ones-stationary matmul that
broadcasts column sums across all partitions, and normalization happens for
free in the PSUM->SBUF copy after the attnV matmul.

The MLP runs entirely in fp8 DoubleRow at 2x PE rate with near-fp16
accuracy via an exact-activation decomposition: each DR pass's two virtual
rows carry (q8(a), q8(a - q8(a))) for ONE 128-deep reduction tile against
duplicated fp8 weights, so the activation quantization error cancels to
second order and only the weight rounding error remains. That weight error
is minimized on the host by adaptive rounding: exact sequential coordinate
descent over the per-element fp8 up/down rounding choices against the
activation Gram matrix, fit on a sampled-batch attention forward pass so
the Grams see (near-)true xm and h statistics. A few trailing MLP2 f-pairs
additionally drop the residual plane and pack two h tiles per DR pass
(plain fp8), spending the remaining error budget for another ~3% of time.

Groups are software-pipelined: group i's attention is followed by group
i-1's MLP1 so the out-projection of group i never waits on the softmax
chain, and MLP2 of group i-1 closes the group. DMA routing matters: big
weight DMAs ride the sync ring (a same-engine DMA issue blocks that
engine's compute queue and with it PSUM-bank release), output stores
alternate over four rings, and the final iteration interleaves the
previous group's MLP1 chains between its attention batches since there is
no next group's QKV to cover those latency chains.
"""

from contextlib import ExitStack

import numpy as np

B, S, E, H, D, F = 384, 128, 512, 4, 128, 2048
NCORES = 8
BL = B // NCORES  # 48 batches per core
GB = 4  # batches per group
NTOK = GB * S  # 512 tokens per group
KE = E // 128  # 4
KF = F // 128  # 16

MMDT_NP = np.float16  # attention matmul operand dtype

_cache: dict = {}


# --------------------------------------------------------------------------
# Workaround: the walrus build in this container accepts at most ONE
# sync-wait command per instruction, while Tile emits several. Hoist every
# extra wait onto its own preceding same-engine InstNoOp (engine queues are
# FIFO, so this is semantically identical).
def _fix_multiwaits(nc):
    import concourse.mybir as mybir

    n = 0
    for fn in nc.m.functions:
        for bb in fn.blocks:
            out = []
            changed = False
            for inst in bb.instructions:
                si = inst.sync_info
                if si is not None and len(si.on_wait) > 1:
                    waits = list(si.on_wait)
                    for w in waits[:-1]:
                        n += 1
                        out.append(
                            mybir.InstNoOp(
                                name=f"I-mwfix-{n}",
                                engine=inst.engine,
                                bass_nofuse=True,
                                sync_info=mybir.SyncInfo(on_wait=[w], on_update=[]),
                            )
                        )
                    inst.sync_info = mybir.SyncInfo(
                        on_wait=[waits[-1]], on_update=list(si.on_update)
                    )
                    changed = True
                out.append(inst)
            if changed:
                bb.instructions = out
    return n


def _build(ng, variant="full", repeat=1, cfg=None, mwfix=True):
    """Build the per-core Bass program processing ng groups of 4 batches."""
    import concourse.bass as bass
    import concourse.mybir as mybir
    import concourse.tile as tile

    cfg = {
        **dict(
            big=5, sz=2, atp=1,
            xtp=3, qkp=2, vp=2, pp=4, rzp=2, atsb=2, tmpp=3, xmp=2, hp=2, yp=3,
            htp=5,
            mlp1_f16=0,  # leading k-tiles of MLP1 kept in fp16 (error knob)
            mlp2_f16=0,  # leading f-tiles of MLP2 kept in fp16 (error knob)
            m1p2=2,  # trailing MLP2 f-pairs as plain fp8 pairs (2 tiles/DR)
            zb=True,  # zero-bias fast path for the h fp8 pair (b1 == 0)
        ),
        **(cfg or {}),
    }
    n1f = cfg["mlp1_f16"]
    n2f = cfg["mlp2_f16"]
    m1p2 = cfg["m1p2"]
    f_m1_start = KF - 2 * m1p2  # f-tiles >= this are plain-fp8 paired
    zb = cfg["zb"]
    gb = GB
    ntok = NTOK
    f32 = mybir.dt.float32
    f16 = mybir.dt.float16
    b16 = mybir.dt.bfloat16
    fp8 = mybir.dt.float8e4
    AF = mybir.ActivationFunctionType
    ALU = mybir.AluOpType
    ts = bass.ts

    ntok_total = BL * S

    nc = bass.Bass("TRN2", target_bir_lowering=False, debug=False)

    xt = nc.dram_tensor("xt", [E, ntok_total], f16, kind="ExternalInput")
    wq_d = nc.dram_tensor("wq", [E, E], f16, kind="ExternalInput")
    wk_d = nc.dram_tensor("wk", [E, E], f16, kind="ExternalInput")
    wv_d = nc.dram_tensor("wv", [E, E], f16, kind="ExternalInput")
    wo_d = nc.dram_tensor("wo", [E, E], f16, kind="ExternalInput")
    w1p_d = nc.dram_tensor("w1p", [128, KE, 2, F], fp8, kind="ExternalInput")
    w2p_d = nc.dram_tensor("w2p", [128, KF, 2, E], fp8, kind="ExternalInput")
    w1_d = nc.dram_tensor("w1", [E, F], f16, kind="ExternalInput")
    w2_d = nc.dram_tensor("w2", [F, E], f16, kind="ExternalInput")
    bias_d = nc.dram_tensor("bias", [128, 32], f32, kind="ExternalInput")
    yt = nc.dram_tensor("yt", [E, ntok_total], f16, kind="ExternalOutput")

    with tile.TileContext(nc) as tc, ExitStack() as ctx:
        singles = ctx.enter_context(tc.tile_pool(name="singles", bufs=1))

        xtp = ctx.enter_context(tc.tile_pool(name="xtp", bufs=cfg["xtp"]))

        def load_weight(name, dram, n_k, width, engine, split=False):
            t = singles.tile([128, n_k, width], f16, tag=f"w_{name}", name=f"w_{name}")
            if split:
                # per-k-tile DMAs: the first matmul of a chain only needs
                # k-tile 0, so compute starts before the full tile lands
                for k in range(n_k):
                    engine.dma_start(
                        out=t[:, k, :], in_=dram[k * 128 : (k + 1) * 128, :]
                    )
            else:
                engine.dma_start(
                    out=t, in_=dram[:, :].rearrange("(k p) w -> p k w", p=128)
                )
            return [t[:, k, :] for k in range(n_k)]

        # group 0's x load is emitted by the pipeline BEFORE these weight
        # loads land on the rings, so first matmuls start early.
        xt_tiles = {}

        def emit_load(i):
            g = i % ng
            c0 = g * ntok
            xt_t = xtp.tile([128, KE, ntok], f16, tag="xt", name="xt_t")
            for k in range(KE):
                nc.sync.dma_start(
                    out=xt_t[:, k, :],
                    in_=xt[k * 128 : (k + 1) * 128, c0 : c0 + ntok],
                )
            xt_tiles[i] = [xt_t[:, k, :] for k in range(KE)]

        emit_load(0)

        # q/k weights ride the gpsimd SWDGE ring, in parallel with group 0's
        # x on the sync ring, so the first matmul starts ~2us earlier.
        # NOTHING may ride the scalar ring early: ACT compute ops (q-copies,
        # exp) queue behind same-engine DMAs and would hold PSUM banks hostage
        wq_sb = load_weight("wq", wq_d, KE, E, nc.gpsimd, split=True)
        wk_sb = load_weight("wk", wk_d, KE, E, nc.gpsimd, split=True)
        # the bias is tiny: its 500ns issue on the scalar ring finishes long
        # before ACT's first compute op, unlike the big weight DMAs
        bias_sb = singles.tile([128, 32], f32, tag="b_all", name="b_all")
        nc.scalar.dma_start(out=bias_sb, in_=bias_d[:, :])
        wv_sb = load_weight("wv", wv_d, KE, E, nc.gpsimd)
        wo_sb = load_weight("wo", wo_d, KE, E, nc.gpsimd)
        # the big fp8 MLP weights ride the sync ring behind the first two
        # x-group loads; they are not needed until MLP1 of group 0 (~25us)
        w1p_sb = singles.tile([128, KE, 2, F], fp8, tag="w_w1p", name="w_w1p")
        w2p_sb = singles.tile([128, KF, 2, E], fp8, tag="w_w2p", name="w_w2p")
        nc.sync.dma_start(out=w1p_sb, in_=w1p_d[:, :, :, :])
        nc.sync.dma_start(out=w2p_sb, in_=w2p_d[:, :, :, :])
        if n1f:
            w1_sb = load_weight("w1", w1_d, KE, F, nc.sync)
        if n2f:
            w2_sb = load_weight("w2", w2_d, KF, E, nc.sync)

        bq_sb = bias_sb[:, 0:KE]
        bk_sb = bias_sb[:, KE : 2 * KE]
        bo_sb = bias_sb[:, 2 * KE : 3 * KE]
        b1_sb = bias_sb[:, 12 : 12 + KF]
        b2_sb = bias_sb[:, 28 : 28 + KE]

        ones = singles.tile([128, 128], b16, tag="ones")
        nc.gpsimd.memset(ones, 1.0)

        qkp = ctx.enter_context(tc.tile_pool(name="qkp", bufs=cfg["qkp"]))
        vp = ctx.enter_context(tc.tile_pool(name="vp", bufs=cfg["vp"]))
        pp = ctx.enter_context(tc.tile_pool(name="pp", bufs=cfg["pp"]))
        rzp = ctx.enter_context(tc.tile_pool(name="rzp", bufs=cfg["rzp"]))
        atp = ctx.enter_context(tc.tile_pool(name="atp", bufs=cfg["atsb"]))
        tmpp = ctx.enter_context(tc.tile_pool(name="tmpp", bufs=cfg["tmpp"]))
        xmp = ctx.enter_context(tc.tile_pool(name="xmp", bufs=cfg["xmp"]))
        hp = ctx.enter_context(tc.tile_pool(name="hp", bufs=cfg["hp"]))
        htp = ctx.enter_context(tc.tile_pool(name="htp", bufs=cfg["htp"]))
        yp = ctx.enter_context(tc.tile_pool(name="yp", bufs=cfg["yp"]))

        ps_big = ctx.enter_context(tc.tile_pool(name="ps_big", bufs=cfg["big"], space="PSUM"))
        ps_sz = ctx.enter_context(tc.tile_pool(name="ps_sz", bufs=cfg["sz"], space="PSUM"))
        ps_atp = ctx.enter_context(tc.tile_pool(name="ps_atp", bufs=cfg["atp"], space="PSUM"))

        n_iters = ng * repeat
        st = {}  # per-iteration state

        def emit_qk(i):
            xt_sb = xt_tiles[i]
            s = st[i] = {}
            q_sb, k_sb = [], []
            for which, w_sb, b_sb, dst in (
                ("q", wq_sb, bq_sb, q_sb),
                ("k", wk_sb, bk_sb, k_sb),
            ):
                for h in range(H):
                    ps = ps_big.tile([128, ntok], f32, tag="big", name="qk_ps")
                    for k in range(KE):
                        nc.tensor.matmul(
                            ps,
                            w_sb[k][:, ts(h, 128)],
                            xt_sb[k],
                            start=(k == 0),
                            stop=(k == KE - 1),
                        )
                    t = qkp.tile([128, ntok], f16, tag=f"{which}{h}", name=f"{which}{h}")
                    # split PSUM-drain ops between ACT (q) and DVE (k)
                    if which == "q":
                        nc.scalar.activation(
                            out=t, in_=ps, func=AF.Identity, bias=b_sb[:, h : h + 1]
                        )
                    else:
                        nc.vector.tensor_scalar_add(t, ps, b_sb[:, h : h + 1])
                    dst.append(t)
            s["q"], s["k"] = q_sb, k_sb

        def emit_v(i):
            s = st[i]
            xt_sb = xt_tiles[i]
            v_sb = []
            for bi in range(gb):
                ps = ps_big.tile([128, E], f32, tag="big", name="v_ps")
                for k in range(KE):
                    nc.tensor.matmul(
                        ps,
                        xt_sb[k][:, ts(bi, 128)],
                        wv_sb[k],
                        start=(k == 0),
                        stop=(k == KE - 1),
                    )
                t = vp.tile([128, E], b16, tag=f"v{bi}", name=f"v{bi}")
                if i == 0:
                    # group 0 has no MLP work in flight: the DVE is idle, and
                    # the ACT queue must reach the first exps quickly
                    nc.vector.tensor_copy(t, ps)
                else:
                    nc.scalar.activation(out=t, in_=ps, func=AF.Identity)
                v_sb.append(t)
            s["v"] = v_sb
            at_t = atp.tile([128, H, ntok], f16, tag="at", name="at_t")
            s["at_t"] = at_t
            s["at"] = [at_t[:, h, :] for h in range(H)]

        def emit_scores(i, bi):
            # S^T[t, (h,s)] for batch bi, then unnormalized exp in bfloat16
            s = st[i]
            s_ps = ps_sz.tile([128, H * 128], f32, tag="sz", name="s_ps")
            for h in range(H):
                nc.tensor.matmul(
                    s_ps[:, ts(h, 128)],
                    s["k"][h][:, ts(bi, 128)],
                    s["q"][h][:, ts(bi, 128)],
                )
            p_sb = pp.tile([128, H * 128], b16, tag="p", name="p_sb")
            nc.scalar.activation(out=p_sb, in_=s_ps, func=AF.Exp)
            s[f"p{bi}"] = p_sb

        def emit_attn_out(i, bi):
            s = st[i]
            p_sb = s.pop(f"p{bi}")
            # column sums of exp, broadcast to all partitions by the
            # ones-stationary matmul
            z_ps = ps_sz.tile([128, H * 128], f32, tag="sz", name="z_ps")
            nc.tensor.matmul(z_ps, ones, p_sb)
            rz_sb = rzp.tile([128, H * 128], f32, tag="rz", name="rz_sb")
            nc.vector.reciprocal(rz_sb, z_ps)
            at_ps = ps_atp.tile([128, H, 128], f32, tag="atp", name="at_ps")
            for h in range(H):
                nc.tensor.matmul(
                    at_ps[:, h, :], s["v"][bi][:, ts(h, 128)], p_sb[:, ts(h, 128)]
                )
            # normalize while copying out of PSUM: at = at_ps * (1/z)
            nc.vector.tensor_mul(
                s["at_t"][:, :, ts(bi, 128)],
                at_ps,
                rz_sb.rearrange("p (h s) -> p h s", h=H),
            )

        def emit_outproj(i):
            s = st[i]
            xt_sb = xt_tiles.pop(i)
            xm_sb = []
            xmx_sb = []
            for m in range(KE):
                ps = ps_big.tile([128, ntok], f32, tag="big", name="o_ps")
                for k in range(KE):
                    nc.tensor.matmul(
                        ps,
                        wo_sb[k][:, ts(m, 128)],
                        s["at"][k],
                        start=(k == 0),
                        stop=(k == KE - 1),
                    )
                xm = xmp.tile([128, ntok], f16, tag=f"xm{m}", name=f"xm{m}")
                if zb:
                    # bo_eff == 0: fold the residual add into the PSUM drain
                    nc.vector.tensor_add(xm, ps, xt_sb[m])
                else:
                    tmp = tmpp.tile([128, ntok], f16, tag="tmp", name="tmp")
                    nc.scalar.activation(
                        out=tmp, in_=ps, func=AF.Identity, bias=bo_sb[:, m : m + 1]
                    )
                    nc.gpsimd.tensor_add(xm, tmp, xt_sb[m])
                xm_sb.append(xm)
                if m >= n1f:
                    # exact-activation DR pair: plane0 = q8(xm),
                    # plane1 = q8(xm - plane0)
                    xx = xmp.tile(
                        [128, 2, ntok], mybir.dt.float8e4, tag=f"xx{m}", name=f"xx{m}"
                    )
                    nc.gpsimd.tensor_copy(xx[:, 0, :], xm)
                    nc.gpsimd.tensor_sub(xx[:, 1, :], xm, xx[:, 0, :])
                    xmx_sb.append(xx)
                else:
                    xmx_sb.append(None)
            s["xm"] = xm_sb
            s["xmx"] = xmx_sb

        def emit_mlp1_chunk(i, fs, last=False, aeng=None):
            s = st[i]
            h_sb = s.setdefault("hx", [])
            for f in fs:
                ps = ps_big.tile([128, ntok], f32, tag="big", name="h_ps")
                for k in range(n1f):
                    nc.tensor.matmul(
                        ps,
                        w1_sb[k][:, ts(f, 128)],
                        s["xm"][k],
                        start=(k == 0),
                        stop=False,
                    )
                for k in range(n1f, KE):
                    nc.tensor.matmul(
                        ps,
                        w1p_sb[:, k, :, ts(f, 128)],
                        s["xmx"][k],
                        start=(k == 0),
                        stop=(k == KE - 1),
                        perf_mode=mybir.MatmulPerfMode.DoubleRow,
                    )
                if f < n2f:
                    # f16 h tile feeding MLP2's fp16 chain
                    t = hp.tile([128, ntok], f16, tag=f"h{f}", name=f"h{f}")
                    if f % 2 == 0:
                        nc.scalar.activation(
                            out=t, in_=ps, func=AF.Relu, bias=b1_sb[:, f : f + 1]
                        )
                    else:
                        nc.vector.tensor_scalar(
                            t, ps, b1_sb[:, f : f + 1], 0.0, op0=ALU.add, op1=ALU.max
                        )
                    h_sb.append(t)
                else:
                    ht = htp.tile([128, ntok], f16, tag="ht", name="ht")
                    if aeng == "dve" or (aeng is None and f % 2):
                        nc.vector.tensor_scalar(
                            ht, ps, b1_sb[:, f : f + 1], 0.0,
                            op0=ALU.add, op1=ALU.max,
                        )
                    else:
                        nc.scalar.activation(
                            out=ht, in_=ps, func=AF.Relu, bias=b1_sb[:, f : f + 1]
                        )
                    if f >= f_m1_start:
                        # plain fp8 pair: two f-tiles share one DR pass
                        if f % 2 == 0:
                            hx = hp.tile(
                                [128, 2, ntok], mybir.dt.float8e4,
                                tag=f"hx{f}", name=f"hx{f}",
                            )
                            h_sb.append(hx)
                        else:
                            hx = h_sb[f - 1]
                            h_sb.append(None)
                        nc.gpsimd.tensor_copy(hx[:, f % 2, :], ht)
                    else:
                        # exact-activation DR pair: one PSUM drain (relu ->
                        # f16), then SBUF-only fp8 pair production on gpsimd
                        hx = hp.tile(
                            [128, 2, ntok], mybir.dt.float8e4,
                            tag=f"hx{f}", name=f"hx{f}",
                        )
                        # the epilogue has no next-group attention to cover
                        # the pair frontier, so split it across two engines
                        peng = nc.vector if (last and f % 2) else nc.gpsimd
                        peng.tensor_copy(hx[:, 0, :], ht)
                        peng.tensor_sub(hx[:, 1, :], ht, hx[:, 0, :])
                        h_sb.append(hx)

        def emit_relu_halved(ps, out, bias_ap):
            # split the PSUM drain across ACT and DVE so the bank frees in
            # half the time (the drain latency gates the PE chain rotation)
            hm = NTOK // 2
            nc.scalar.activation(
                out=out[:, :hm], in_=ps[:, :hm], func=AF.Relu, bias=bias_ap
            )
            nc.vector.tensor_scalar(
                out[:, hm:], ps[:, hm:], bias_ap, 0.0, op0=ALU.add, op1=ALU.max
            )

        def emit_mlp2_store(i, nq=2):
            s = st[i]
            g = i % ng
            c0 = g * ntok
            h_sb = s["hx"]
            yf = yp.tile([128, KE, ntok], f16, tag="yf", name="yf")
            for m in range(KE):
                ps = ps_big.tile([128, ntok], f32, tag="big", name="acc_ps")
                tmpf = None
                for f in range(n2f):
                    nc.tensor.matmul(
                        ps,
                        w2_sb[f][:, ts(m, 128)],
                        h_sb[f],
                        start=(f == 0),
                        stop=False,
                    )
                for f in range(n2f, KF):
                    if f >= f_m1_start:
                        if f % 2:
                            continue  # odd tile rides its pair's DR pass
                        # plain pair: planes step over adjacent f-tiles
                        stat = w2p_sb[:, f : f + 2, 0, ts(m, 128)]
                    else:
                        stat = w2p_sb[:, f, :, ts(m, 128)]
                    nc.tensor.matmul(
                        ps,
                        stat,
                        h_sb[f],
                        start=(f == 0),
                        stop=(f == KF - 1 or f == f_m1_start + 2 * m1p2 - 2),
                        perf_mode=mybir.MatmulPerfMode.DoubleRow,
                    )
                if not zb:
                    tmpf = tmpp.tile([128, ntok], f32, tag="tmpf", name="tmpf")
                # half-width copy/add/store chain: the store of one half
                # overlaps the residual add of the other, shortening the
                # post-PE drain at the end of the program
                for half in range(nq):
                    hs = slice(half * (ntok // nq), (half + 1) * (ntok // nq))
                    if zb:
                        # b2 == 0: drain + scale + residual in one DVE op
                        nc.vector.scalar_tensor_tensor(
                            yf[:, m, hs], ps[:, hs], 1.0 / 256.0,
                            s["xm"][m][:, hs], op0=ALU.mult, op1=ALU.add,
                        )
                    else:
                        nc.scalar.activation(
                            out=tmpf[:, hs], in_=ps[:, hs], func=AF.Identity,
                            scale=1.0 / 256.0, bias=b2_sb[:, m : m + 1],
                        )
                        nc.gpsimd.tensor_add(
                            yf[:, m, hs], tmpf[:, hs], s["xm"][m][:, hs]
                        )
                    # alternate store rings: a same-engine DMA issue occupies
                    # its queue for ~500ns and would stall compute ops
                    dmae = (nc.scalar, nc.sync, nc.gpsimd, nc.sync)[
                        (2 * m + half) % 4
                    ]
                    dmae.dma_start(
                        out=yt[m * 128 : (m + 1) * 128, c0 + half * (ntok // nq) : c0 + (half + 1) * (ntok // nq)],
                        in_=yf[:, m, hs],
                    )
            del st[i]

        # q/k/v of group i+1 are hoisted into group i's slot: they are pure PE
        # work with no softmax dependencies, so they cover the DVE-bound
        # normalize chain ahead of oproj even on the first groups (ramp).
        if n_iters > 1:
            emit_load(1)
        emit_qk(0)
        emit_v(0)
        for i in range(n_iters):
            if i + 2 < n_iters:
                emit_load(i + 2)
            if i == n_iters - 1 and i >= 1:
                # last slot has no next-group projections to cover the
                # scores->exp->attnv latency chains; interleave the previous
                # group's MLP1 chains instead
                for bi in range(gb):
                    emit_scores(i, bi)
                    emit_mlp1_chunk(i - 1, range(bi * 2, bi * 2 + 2))
                for bi in range(gb):
                    emit_attn_out(i, bi)
                    emit_mlp1_chunk(i - 1, range(8 + bi * 2, 8 + bi * 2 + 2))
                emit_outproj(i)
                emit_mlp2_store(i - 1)
                emit_mlp1_chunk(i, range(KF), last=True)
                emit_mlp2_store(i)
            else:
                for bi in range(gb):
                    emit_scores(i, bi)
                # next group's projections sit between scores and attn-outs so
                # their DVE bias-adds run ahead of the recip/normalize backlog
                # (the v chains' PSUM ring waits on those bias-adds)
                if i + 1 < n_iters:
                    emit_qk(i + 1)
                    emit_v(i + 1)
                for bi in range(gb):
                    emit_attn_out(i, bi)
                if i >= 1:
                    emit_mlp1_chunk(i - 1, range(KF))
                emit_outproj(i)
                if i >= 1:
                    emit_mlp2_store(i - 1)
        if n_iters == 1:
            emit_mlp1_chunk(0, range(KF), last=True)
            emit_mlp2_store(0)

    if mwfix:
        _fix_multiwaits(nc)
    return nc


def _get_program(ng, variant="full", repeat=1, cfg=None):
    key = ("nc", ng, variant, repeat, tuple(sorted((cfg or {}).items())))
    if key not in _cache:
        _cache[key] = _build(ng, variant, repeat, cfg)
    return _cache[key]


# --------------------------------------------------------------------------
# Host-side adaptive rounding of the fp8 MLP weights: minimize
# ||acts @ (w_opt - w)||_F over the per-element choice between the two
# neighboring fp8 grid points, by exact sequential coordinate descent
# (vectorized across output columns).
def _fp8_neighbor_toward(w, wq):
    import ml_dtypes

    b = wq.astype(ml_dtypes.float8_e4m3fn).view(np.uint8)
    sign = (b & 0x80) != 0
    mag = (b & 0x7F).astype(np.int16)
    d = w - wq
    up = d > 0
    newmag = np.where(up ^ sign, mag + 1, mag - 1)
    flip = newmag < 0
    newmag = np.where(flip, 1, newmag)
    newsign = sign ^ flip
    newmag = np.clip(newmag, 0, 0x7E)
    out = newmag.astype(np.uint8) | np.where(newsign, 0x80, 0).astype(np.uint8)
    res = out.view(ml_dtypes.float8_e4m3fn).astype(np.float32)
    return np.where(d == 0, wq, res)


def _q8(a):
    import ml_dtypes

    return np.asarray(a, np.float32).astype(ml_dtypes.float8_e4m3fn).astype(np.float32)


def _adaround(w, acts, n_sweeps=4):
    a = np.ascontiguousarray(acts, np.float32)
    G = (a.T @ a) / len(a)
    wq = _q8(w)
    alt = _fp8_neighbor_toward(w, wq)
    cur = wq.copy()
    K = w.shape[0]
    delta = cur - w
    g = G @ delta
    Gd = G.diagonal()
    for _ in range(n_sweeps):
        nflip = 0
        for i in range(K):
            other = np.where(cur[i] == wq[i], alt[i], wq[i])
            d = other - cur[i]
            gain = 2 * d * g[i] + d * d * Gd[i]
            m = gain < -1e-14
            if m.any():
                du = np.where(m, d, 0.0)
                cur[i] += du
                g += np.outer(G[:, i], du)
                nflip += int(m.sum())
        if nflip == 0:
            break
    return cur


def make_in_maps(x, wq, bq, wk, bk, wv, bv, wo, bo, w1, b1, w2, b2,
                 ada_sweeps=4, ada_sample=3072):
    import ml_dtypes

    x = np.asarray(x, np.float32)
    to_16 = lambda a: np.ascontiguousarray(np.asarray(a, np.float32).astype(MMDT_NP))

    # host-side prep: shard + transpose + cast
    ntok_total = BL * S
    x_sh = x.reshape(NCORES, ntok_total, E)
    xts = [np.ascontiguousarray(x_sh[c].T).astype(MMDT_NP) for c in range(NCORES)]

    # MLP scale convention: w1 (and b1) carry x16 so the fp8 weights sit in
    # e4m3's normal range; h tiles then hold 16*h and w2 carries x16 too, so
    # the MLP2 PSUM result is 256x and one scale of 1/256 restores it.
    # Both scalings are exact powers of two.
    w1_s = np.asarray(w1, np.float32) * 16.0
    w2_s = np.asarray(w2, np.float32) * 16.0
    b1_s = np.asarray(b1, np.float32) * 16.0

    # --- adaptive rounding of the fp8 weights, fit on a sampled-batch
    # attention forward pass so the Gram matrices see the true MLP inputs ---
    q16_ = lambda a: a.astype(np.float16).astype(np.float32)
    nbs = max(1, ada_sample // S)
    xs = x.reshape(B, S, E)[:: max(1, B // nbs)][:nbs]  # [nbs, S, E]
    xs16 = q16_(xs.reshape(nbs * S, E))
    wq16, wk16, wv16, wo16 = (
        q16_(np.asarray(w, np.float32)) for w in (wq, wk, wv, wo)
    )
    bo_eff_f = (
        np.asarray(bv, np.float64) @ np.asarray(wo, np.float64)
        + np.asarray(bo, np.float64)
    ).astype(np.float32)
    qs = q16_(xs16 @ wq16 + np.asarray(bq, np.float32))
    ks = q16_(xs16 @ wk16 + np.asarray(bk, np.float32))
    vs = xs16 @ wv16
    qh = qs.reshape(nbs, S, H, D)
    kh = ks.reshape(nbs, S, H, D)
    vh = vs.reshape(nbs, S, H, D)
    sc = np.einsum("bshd,bthd->bhst", qh, kh, optimize=True)
    sc -= sc.max(axis=-1, keepdims=True)
    p = np.exp(sc, dtype=np.float32)
    p /= p.sum(axis=-1, keepdims=True)
    attn = np.einsum("bhst,bthd->bshd", p, vh, optimize=True)
    at_s = q16_(attn.reshape(nbs * S, E))
    xm_s = q16_(q16_(at_s @ wo16 + bo_eff_f) + xs16)

    x8 = _q8(xm_s)
    axx = x8 + _q8(xm_s - x8)
    if ada_sweeps > 0:
        w1opt = _adaround(w1_s, axx, ada_sweeps)
    else:
        w1opt = _q8(w1_s)
    hprox = np.maximum(axx @ w1opt + b1_s, 0.0)
    h8 = _q8(hprox)
    ahh = h8 + _q8(hprox.astype(np.float16).astype(np.float32) - h8)
    if ada_sweeps > 0:
        w2opt = _adaround(w2_s, ahh, ada_sweeps)
    else:
        w2opt = _q8(w2_s)

    # dual-plane stationary layout [128, K, 2, M]: both planes carry the
    # same (ada-rounded) fp8 weights; the moving pair holds (q8(a), resid)
    w1pairs = np.ascontiguousarray(
        np.broadcast_to(
            w1opt.reshape(KE, 128, 1, F).transpose(1, 0, 2, 3), (128, KE, 2, F)
        )
    ).astype(ml_dtypes.float8_e4m3)
    w2pairs = np.ascontiguousarray(
        np.broadcast_to(
            w2opt.reshape(KF, 128, 1, E).transpose(1, 0, 2, 3), (128, KF, 2, E)
        )
    ).astype(ml_dtypes.float8_e4m3)

    wq_b, wk_b, wv_b, wo_b, w1_b, w2_b = map(
        to_16, (wq, wk, wv, wo, w1_s, w2_s)
    )

    resh = lambda b, nk: np.asarray(b, np.float32).reshape(nk, 128).T
    # bv is folded into the output-projection bias: P rows sum to 1, so
    # attn@wo + bo == (P@v_nobias)@wo + (bv@wo + bo).
    bo_eff = (
        np.asarray(bv, np.float64) @ np.asarray(wo, np.float64)
        + np.asarray(bo, np.float64)
    ).astype(np.float32)
    bias_pack = np.zeros((128, 32), np.float32)
    bias_pack[:, 0:KE] = resh(bq, KE)
    bias_pack[:, KE : 2 * KE] = resh(bk, KE)
    bias_pack[:, 2 * KE : 3 * KE] = resh(bo_eff, KE)
    bias_pack[:, 12 : 12 + KF] = resh(b1_s, KF)
    bias_pack[:, 28 : 28 + KE] = resh(b2, KE)

    in_maps = []
    for c in range(NCORES):
        in_maps.append(
            {
                "xt": xts[c],
                "wq": wq_b,
                "wk": wk_b,
                "wv": wv_b,
                "wo": wo_b,
                "w1p": w1pairs,
                "w2p": w2pairs,
                "w1": w1_b,
                "w2": w2_b,
                "bias": bias_pack,
            }
        )
    return in_maps


def kernel(
    x, wq, bq, wk, bk, wv, bv, wo, bo, w1, b1, w2, b2, _ng=BL // GB, _cfg=None
):
    import os

    from concourse.bass_utils import run_bass_kernel_spmd

    # The NTFF trace hook module does not exist in this container; make sure
    # run_bass_kernel_spmd never takes the trace branch even if BASS_TRACE
    # is set in the environment.
    os.environ["BASS_NEVER_TRACE"] = "1"

    in_maps = make_in_maps(x, wq, bq, wk, bk, wv, bv, wo, bo, w1, b1, w2, b2)
    ntok_total = BL * S
    cfg = dict(_cfg or {})
    # the zb fast paths fold bo_eff (= bv @ wo + bo) and b2 into fused
    # drain+residual ops; they require those biases to be exactly zero
    bo_eff = np.asarray(bv, np.float64) @ np.asarray(wo, np.float64) + np.asarray(
        bo, np.float64
    )
    if not (np.all(bo_eff == 0.0) and np.all(np.asarray(b2) == 0.0)):
        cfg["zb"] = False
    nc = _get_program(_ng, cfg=cfg)

    res = run_bass_kernel_spmd(nc, in_maps, core_ids=list(range(NCORES)))
    _cache["last_result"] = res

    out = np.empty((NCORES, ntok_total, E), np.float32)
    for c in range(NCORES):
        out[c] = res.results[c]["yt"].T.astype(np.float32)
    return out.reshape(B, S, E)


# revision 51
# speedup vs baseline: 1.2108x; 1.0037x over previous
"""Trainium2 Bass kernel for a dense transformer block (attention + MLP).

Strategy: data-parallel over batch across 8 NeuronCores (48 batches each).
Per core, batches are processed in groups of 4 (512 tokens) so every dense
matmul has a 512-wide moving operand. Activations live transposed in SBUF
([feature, token]) so DRAM-layout weights serve directly as the stationary
matmul operand. Attention matmul operands are fp16 (full PE rate, 10-bit
mantissa); accumulation is fp32 in PSUM.

Attention is computed k-major to avoid PE transposes entirely: scores are
built as S^T = K_h^T-stationary x Q_h-moving giving [t, s] tiles, exp is
taken UNnormalized into bfloat16 (fp32-range exponent, so no max-subtraction
is needed), the softmax denominators come from a ones-stationary matmul that
broadcasts column sums across all partitions, and normalization happens for
free in the PSUM->SBUF copy after the attnV matmul.

The MLP runs entirely in fp8 DoubleRow at 2x PE rate with near-fp16
accuracy via an exact-activation decomposition: each DR pass's two virtual
rows carry (q8(a), q8(a - q8(a))) for ONE 128-deep reduction tile against
duplicated fp8 weights, so the activation quantization error cancels to
second order and only the weight rounding error remains. That weight error
is minimized on the host by adaptive rounding: exact sequential coordinate
descent over the per-element fp8 up/down rounding choices against the
activation Gram matrix, fit on a sampled-batch attention forward pass so
the Grams see (near-)true xm and h statistics. A few trailing MLP2 f-pairs
additionally drop the residual plane and pack two h tiles per DR pass
(plain fp8), spending the remaining error budget for another ~3% of time.

Groups are software-pipelined: group i's attention is followed by group
i-1's MLP1 so the out-projection of group i never waits on the softmax
chain, and MLP2 of group i-1 closes the group. DMA routing matters: big
weight DMAs ride the sync ring (a same-engine DMA issue blocks that
engine's compute queue and with it PSUM-bank release), output stores
alternate over four rings, and the final iteration interleaves the
previous group's MLP1 chains between its attention batches since there is
no next group's QKV to cover those latency chains.
"""

from contextlib import ExitStack

import numpy as np

B, S, E, H, D, F = 384, 128, 512, 4, 128, 2048
NCORES = 8
BL = B // NCORES  # 48 batches per core
GB = 4  # batches per group
NTOK = GB * S  # 512 tokens per group
KE = E // 128  # 4
KF = F // 128  # 16

MMDT_NP = np.float16  # attention matmul operand dtype

_cache: dict = {}


# --------------------------------------------------------------------------
# Workaround: the walrus build in this container accepts at most ONE
# sync-wait command per instruction, while Tile emits several. Hoist every
# extra wait onto its own preceding same-engine InstNoOp (engine queues are
# FIFO, so this is semantically identical).
def _fix_multiwaits(nc):
    import concourse.mybir as mybir

    n = 0
    for fn in nc.m.functions:
        for bb in fn.blocks:
            out = []
            changed = False
            for inst in bb.instructions:
                si = inst.sync_info
                if si is not None and len(si.on_wait) > 1:
                    waits = list(si.on_wait)
                    for w in waits[:-1]:
                        n += 1
                        out.append(
                            mybir.InstNoOp(
                                name=f"I-mwfix-{n}",
                                engine=inst.engine,
                                bass_nofuse=True,
                                sync_info=mybir.SyncInfo(on_wait=[w], on_update=[]),
                            )
                        )
                    inst.sync_info = mybir.SyncInfo(
                        on_wait=[waits[-1]], on_update=list(si.on_update)
                    )
                    changed = True
                out.append(inst)
            if changed:
                bb.instructions = out
    return n


def _build(ng, variant="full", repeat=1, cfg=None, mwfix=True):
    """Build the per-core Bass program processing ng groups of 4 batches."""
    import concourse.bass as bass
    import concourse.mybir as mybir
    import concourse.tile as tile

    cfg = {
        **dict(
            big=5, sz=2, atp=1,
            xtp=3, qkp=2, vp=2, pp=3, rzp=2, atsb=2, tmpp=3, xmp=2, hp=2, yp=3,
            htp=5,
            mlp1_f16=0,  # leading k-tiles of MLP1 kept in fp16 (error knob)
            mlp2_f16=0,  # leading f-tiles of MLP2 kept in fp16 (error knob)
            m1p2=2,  # trailing MLP2 f-pairs as plain fp8 pairs (2 tiles/DR)
            zb=True,  # zero-bias fast path for the h fp8 pair (b1 == 0)
        ),
        **(cfg or {}),
    }
    n1f = cfg["mlp1_f16"]
    n2f = cfg["mlp2_f16"]
    m1p2 = cfg["m1p2"]
    f_m1_start = KF - 2 * m1p2  # f-tiles >= this are plain-fp8 paired
    zb = cfg["zb"]
    gb = GB
    ntok = NTOK
    f32 = mybir.dt.float32
    f16 = mybir.dt.float16
    b16 = mybir.dt.bfloat16
    fp8 = mybir.dt.float8e4
    AF = mybir.ActivationFunctionType
    ALU = mybir.AluOpType
    ts = bass.ts

    ntok_total = BL * S

    nc = bass.Bass("TRN2", target_bir_lowering=False, debug=False)

    xt = nc.dram_tensor("xt", [E, ntok_total], f16, kind="ExternalInput")
    wq_d = nc.dram_tensor("wq", [E, E], f16, kind="ExternalInput")
    wk_d = nc.dram_tensor("wk", [E, E], f16, kind="ExternalInput")
    wv_d = nc.dram_tensor("wv", [E, E], f16, kind="ExternalInput")
    wo_d = nc.dram_tensor("wo", [E, E], f16, kind="ExternalInput")
    w1p_d = nc.dram_tensor("w1p", [128, KE, 2, F], fp8, kind="ExternalInput")
    w2p_d = nc.dram_tensor("w2p", [128, KF, 2, E], fp8, kind="ExternalInput")
    w1_d = nc.dram_tensor("w1", [E, F], f16, kind="ExternalInput")
    w2_d = nc.dram_tensor("w2", [F, E], f16, kind="ExternalInput")
    bias_d = nc.dram_tensor("bias", [128, 32], f32, kind="ExternalInput")
    yt = nc.dram_tensor("yt", [E, ntok_total], f16, kind="ExternalOutput")

    with tile.TileContext(nc) as tc, ExitStack() as ctx:
        singles = ctx.enter_context(tc.tile_pool(name="singles", bufs=1))

        xtp = ctx.enter_context(tc.tile_pool(name="xtp", bufs=cfg["xtp"]))

        def load_weight(name, dram, n_k, width, engine, split=False):
            t = singles.tile([128, n_k, width], f16, tag=f"w_{name}", name=f"w_{name}")
            if split:
                # per-k-tile DMAs: the first matmul of a chain only needs
                # k-tile 0, so compute starts before the full tile lands
                for k in range(n_k):
                    engine.dma_start(
                        out=t[:, k, :], in_=dram[k * 128 : (k + 1) * 128, :]
                    )
            else:
                engine.dma_start(
                    out=t, in_=dram[:, :].rearrange("(k p) w -> p k w", p=128)
                )
            return [t[:, k, :] for k in range(n_k)]

        # group 0's x load is emitted by the pipeline BEFORE these weight
        # loads land on the rings, so first matmuls start early.
        xt_tiles = {}

        def emit_load(i):
            g = i % ng
            c0 = g * ntok
            xt_t = xtp.tile([128, KE, ntok], f16, tag="xt", name="xt_t")
            for k in range(KE):
                nc.sync.dma_start(
                    out=xt_t[:, k, :],
                    in_=xt[k * 128 : (k + 1) * 128, c0 : c0 + ntok],
                )
            xt_tiles[i] = [xt_t[:, k, :] for k in range(KE)]

        emit_load(0)

        # q/k weights ride the gpsimd SWDGE ring, in parallel with group 0's
        # x on the sync ring, so the first matmul starts ~2us earlier.
        # NOTHING may ride the scalar ring early: ACT compute ops (q-copies,
        # exp) queue behind same-engine DMAs and would hold PSUM banks hostage
        wq_sb = load_weight("wq", wq_d, KE, E, nc.gpsimd, split=True)
        wk_sb = load_weight("wk", wk_d, KE, E, nc.gpsimd, split=True)
        # the bias is tiny: its 500ns issue on the scalar ring finishes long
        # before ACT's first compute op, unlike the big weight DMAs
        bias_sb = singles.tile([128, 32], f32, tag="b_all", name="b_all")
        nc.scalar.dma_start(out=bias_sb, in_=bias_d[:, :])
        wv_sb = load_weight("wv", wv_d, KE, E, nc.gpsimd)
        wo_sb = load_weight("wo", wo_d, KE, E, nc.gpsimd)
        # the big fp8 MLP weights ride the sync ring behind the first two
        # x-group loads; they are not needed until MLP1 of group 0 (~25us)
        w1p_sb = singles.tile([128, KE, 2, F], fp8, tag="w_w1p", name="w_w1p")
        w2p_sb = singles.tile([128, KF, 2, E], fp8, tag="w_w2p", name="w_w2p")
        nc.sync.dma_start(out=w1p_sb, in_=w1p_d[:, :, :, :])
        nc.sync.dma_start(out=w2p_sb, in_=w2p_d[:, :, :, :])
        if n1f:
            w1_sb = load_weight("w1", w1_d, KE, F, nc.sync)
        if n2f:
            w2_sb = load_weight("w2", w2_d, KF, E, nc.sync)

        bq_sb = bias_sb[:, 0:KE]
        bk_sb = bias_sb[:, KE : 2 * KE]
        bo_sb = bias_sb[:, 2 * KE : 3 * KE]
        b1_sb = bias_sb[:, 12 : 12 + KF]
        b2_sb = bias_sb[:, 28 : 28 + KE]

        ones = singles.tile([128, 128], b16, tag="ones")
        nc.gpsimd.memset(ones, 1.0)

        qkp = ctx.enter_context(tc.tile_pool(name="qkp", bufs=cfg["qkp"]))
        vp = ctx.enter_context(tc.tile_pool(name="vp", bufs=cfg["vp"]))
        pp = ctx.enter_context(tc.tile_pool(name="pp", bufs=cfg["pp"]))
        rzp = ctx.enter_context(tc.tile_pool(name="rzp", bufs=cfg["rzp"]))
        atp = ctx.enter_context(tc.tile_pool(name="atp", bufs=cfg["atsb"]))
        tmpp = ctx.enter_context(tc.tile_pool(name="tmpp", bufs=cfg["tmpp"]))
        xmp = ctx.enter_context(tc.tile_pool(name="xmp", bufs=cfg["xmp"]))
        hp = ctx.enter_context(tc.tile_pool(name="hp", bufs=cfg["hp"]))
        htp = ctx.enter_context(tc.tile_pool(name="htp", bufs=cfg["htp"]))
        yp = ctx.enter_context(tc.tile_pool(name="yp", bufs=cfg["yp"]))

        ps_big = ctx.enter_context(tc.tile_pool(name="ps_big", bufs=cfg["big"], space="PSUM"))
        ps_sz = ctx.enter_context(tc.tile_pool(name="ps_sz", bufs=cfg["sz"], space="PSUM"))
        ps_atp = ctx.enter_context(tc.tile_pool(name="ps_atp", bufs=cfg["atp"], space="PSUM"))

        n_iters = ng * repeat
        st = {}  # per-iteration state

        def emit_qk(i):
            xt_sb = xt_tiles[i]
            s = st[i] = {}
            q_sb, k_sb = [], []
            for which, w_sb, b_sb, dst in (
                ("q", wq_sb, bq_sb, q_sb),
                ("k", wk_sb, bk_sb, k_sb),
            ):
                for h in range(H):
                    ps = ps_big.tile([128, ntok], f32, tag="big", name="qk_ps")
                    for k in range(KE):
                        nc.tensor.matmul(
                            ps,
                            w_sb[k][:, ts(h, 128)],
                            xt_sb[k],
                            start=(k == 0),
                            stop=(k == KE - 1),
                        )
                    t = qkp.tile([128, ntok], f16, tag=f"{which}{h}", name=f"{which}{h}")
                    # split PSUM-drain ops between ACT (q) and DVE (k)
                    if which == "q":
                        nc.scalar.activation(
                            out=t, in_=ps, func=AF.Identity, bias=b_sb[:, h : h + 1]
                        )
                    else:
                        nc.vector.tensor_scalar_add(t, ps, b_sb[:, h : h + 1])
                    dst.append(t)
            s["q"], s["k"] = q_sb, k_sb

        def emit_v(i):
            s = st[i]
            xt_sb = xt_tiles[i]
            v_sb = []
            for bi in range(gb):
                ps = ps_big.tile([128, E], f32, tag="big", name="v_ps")
                for k in range(KE):
                    nc.tensor.matmul(
                        ps,
                        xt_sb[k][:, ts(bi, 128)],
                        wv_sb[k],
                        start=(k == 0),
                        stop=(k == KE - 1),
                    )
                t = vp.tile([128, E], b16, tag=f"v{bi}", name=f"v{bi}")
                if i == 0:
                    # group 0 has no MLP work in flight: the DVE is idle, and
                    # the ACT queue must reach the first exps quickly
                    nc.vector.tensor_copy(t, ps)
                else:
                    nc.scalar.activation(out=t, in_=ps, func=AF.Identity)
                v_sb.append(t)
            s["v"] = v_sb
            at_t = atp.tile([128, H, ntok], f16, tag="at", name="at_t")
            s["at_t"] = at_t
            s["at"] = [at_t[:, h, :] for h in range(H)]

        def emit_scores(i, bi):
            # S^T[t, (h,s)] for batch bi, then unnormalized exp in bfloat16
            s = st[i]
            s_ps = ps_sz.tile([128, H * 128], f32, tag="sz", name="s_ps")
            for h in range(H):
                nc.tensor.matmul(
                    s_ps[:, ts(h, 128)],
                    s["k"][h][:, ts(bi, 128)],
                    s["q"][h][:, ts(bi, 128)],
                )
            p_sb = pp.tile([128, H * 128], b16, tag="p", name="p_sb")
            nc.scalar.activation(out=p_sb, in_=s_ps, func=AF.Exp)
            s[f"p{bi}"] = p_sb

        def emit_attn_out(i, bi):
            s = st[i]
            p_sb = s.pop(f"p{bi}")
            # column sums of exp, broadcast to all partitions by the
            # ones-stationary matmul
            z_ps = ps_sz.tile([128, H * 128], f32, tag="sz", name="z_ps")
            nc.tensor.matmul(z_ps, ones, p_sb)
            rz_sb = rzp.tile([128, H * 128], f32, tag="rz", name="rz_sb")
            nc.vector.reciprocal(rz_sb, z_ps)
            at_ps = ps_atp.tile([128, H, 128], f32, tag="atp", name="at_ps")
            for h in range(H):
                nc.tensor.matmul(
                    at_ps[:, h, :], s["v"][bi][:, ts(h, 128)], p_sb[:, ts(h, 128)]
                )
            # normalize while copying out of PSUM: at = at_ps * (1/z)
            nc.vector.tensor_mul(
                s["at_t"][:, :, ts(bi, 128)],
                at_ps,
                rz_sb.rearrange("p (h s) -> p h s", h=H),
            )

        def emit_outproj(i):
            s = st[i]
            xt_sb = xt_tiles.pop(i)
            xm_sb = []
            xmx_sb = []
            for m in range(KE):
                ps = ps_big.tile([128, ntok], f32, tag="big", name="o_ps")
                for k in range(KE):
                    nc.tensor.matmul(
                        ps,
                        wo_sb[k][:, ts(m, 128)],
                        s["at"][k],
                        start=(k == 0),
                        stop=(k == KE - 1),
                    )
                xm = xmp.tile([128, ntok], f16, tag=f"xm{m}", name=f"xm{m}")
                if zb:
                    # bo_eff == 0: fold the residual add into the PSUM drain
                    nc.vector.tensor_add(xm, ps, xt_sb[m])
                else:
                    tmp = tmpp.tile([128, ntok], f16, tag="tmp", name="tmp")
                    nc.scalar.activation(
                        out=tmp, in_=ps, func=AF.Identity, bias=bo_sb[:, m : m + 1]
                    )
                    nc.gpsimd.tensor_add(xm, tmp, xt_sb[m])
                xm_sb.append(xm)
                if m >= n1f:
                    # exact-activation DR pair: plane0 = q8(xm),
                    # plane1 = q8(xm - plane0)
                    xx = xmp.tile(
                        [128, 2, ntok], mybir.dt.float8e4, tag=f"xx{m}", name=f"xx{m}"
                    )
                    nc.gpsimd.tensor_copy(xx[:, 0, :], xm)
                    nc.gpsimd.tensor_sub(xx[:, 1, :], xm, xx[:, 0, :])
                    xmx_sb.append(xx)
                else:
                    xmx_sb.append(None)
            s["xm"] = xm_sb
            s["xmx"] = xmx_sb

        def emit_mlp1_chunk(i, fs, last=False, aeng=None):
            s = st[i]
            h_sb = s.setdefault("hx", [])
            for f in fs:
                ps = ps_big.tile([128, ntok], f32, tag="big", name="h_ps")
                for k in range(n1f):
                    nc.tensor.matmul(
                        ps,
                        w1_sb[k][:, ts(f, 128)],
                        s["xm"][k],
                        start=(k == 0),
                        stop=False,
                    )
                for k in range(n1f, KE):
                    nc.tensor.matmul(
                        ps,
                        w1p_sb[:, k, :, ts(f, 128)],
                        s["xmx"][k],
                        start=(k == 0),
                        stop=(k == KE - 1),
                        perf_mode=mybir.MatmulPerfMode.DoubleRow,
                    )
                if f < n2f:
                    # f16 h tile feeding MLP2's fp16 chain
                    t = hp.tile([128, ntok], f16, tag=f"h{f}", name=f"h{f}")
                    if f % 2 == 0:
                        nc.scalar.activation(
                            out=t, in_=ps, func=AF.Relu, bias=b1_sb[:, f : f + 1]
                        )
                    else:
                        nc.vector.tensor_scalar(
                            t, ps, b1_sb[:, f : f + 1], 0.0, op0=ALU.add, op1=ALU.max
                        )
                    h_sb.append(t)
                else:
                    ht = htp.tile([128, ntok], f16, tag="ht", name="ht")
                    if aeng == "dve" or (aeng is None and f % 2):
                        nc.vector.tensor_scalar(
                            ht, ps, b1_sb[:, f : f + 1], 0.0,
                            op0=ALU.add, op1=ALU.max,
                        )
                    else:
                        nc.scalar.activation(
                            out=ht, in_=ps, func=AF.Relu, bias=b1_sb[:, f : f + 1]
                        )
                    if f >= f_m1_start:
                        # plain fp8 pair: two f-tiles share one DR pass
                        if f % 2 == 0:
                            hx = hp.tile(
                                [128, 2, ntok], mybir.dt.float8e4,
                                tag=f"hx{f}", name=f"hx{f}",
                            )
                            h_sb.append(hx)
                        else:
                            hx = h_sb[f - 1]
                            h_sb.append(None)
                        nc.gpsimd.tensor_copy(hx[:, f % 2, :], ht)
                    else:
                        # exact-activation DR pair: one PSUM drain (relu ->
                        # f16), then SBUF-only fp8 pair production on gpsimd
                        hx = hp.tile(
                            [128, 2, ntok], mybir.dt.float8e4,
                            tag=f"hx{f}", name=f"hx{f}",
                        )
                        # the epilogue has no next-group attention to cover
                        # the pair frontier, so split it across two engines
                        peng = nc.vector if (last and f % 2) else nc.gpsimd
                        peng.tensor_copy(hx[:, 0, :], ht)
                        peng.tensor_sub(hx[:, 1, :], ht, hx[:, 0, :])
                        h_sb.append(hx)

        def emit_relu_halved(ps, out, bias_ap):
            # split the PSUM drain across ACT and DVE so the bank frees in
            # half the time (the drain latency gates the PE chain rotation)
            hm = NTOK // 2
            nc.scalar.activation(
                out=out[:, :hm], in_=ps[:, :hm], func=AF.Relu, bias=bias_ap
            )
            nc.vector.tensor_scalar(
                out[:, hm:], ps[:, hm:], bias_ap, 0.0, op0=ALU.add, op1=ALU.max
            )

        def emit_mlp2_store(i, nq=2):
            s = st[i]
            g = i % ng
            c0 = g * ntok
            h_sb = s["hx"]
            yf = yp.tile([128, KE, ntok], f16, tag="yf", name="yf")
            for m in range(KE):
                ps = ps_big.tile([128, ntok], f32, tag="big", name="acc_ps")
                tmpf = None
                for f in range(n2f):
                    nc.tensor.matmul(
                        ps,
                        w2_sb[f][:, ts(m, 128)],
                        h_sb[f],
                        start=(f == 0),
                        stop=False,
                    )
                for f in range(n2f, KF):
                    if f >= f_m1_start:
                        if f % 2:
                            continue  # odd tile rides its pair's DR pass
                        # plain pair: planes step over adjacent f-tiles
                        stat = w2p_sb[:, f : f + 2, 0, ts(m, 128)]
                    else:
                        stat = w2p_sb[:, f, :, ts(m, 128)]
                    nc.tensor.matmul(
                        ps,
                        stat,
                        h_sb[f],
                        start=(f == 0),
                        stop=(f == KF - 1 or f == f_m1_start + 2 * m1p2 - 2),
                        perf_mode=mybir.MatmulPerfMode.DoubleRow,
                    )
                if not zb:
                    tmpf = tmpp.tile([128, ntok], f32, tag="tmpf", name="tmpf")
                # half-width copy/add/store chain: the store of one half
                # overlaps the residual add of the other, shortening the
                # post-PE drain at the end of the program
                for half in range(nq):
                    hs = slice(half * (ntok // nq), (half + 1) * (ntok // nq))
                    if zb:
                        # b2 == 0: drain + scale + residual in one DVE op
                        nc.vector.scalar_tensor_tensor(
                            yf[:, m, hs], ps[:, hs], 1.0 / 256.0,
                            s["xm"][m][:, hs], op0=ALU.mult, op1=ALU.add,
                        )
                    else:
                        nc.scalar.activation(
                            out=tmpf[:, hs], in_=ps[:, hs], func=AF.Identity,
                            scale=1.0 / 256.0, bias=b2_sb[:, m : m + 1],
                        )
                        nc.gpsimd.tensor_add(
                            yf[:, m, hs], tmpf[:, hs], s["xm"][m][:, hs]
                        )
                    # alternate store rings: a same-engine DMA issue occupies
                    # its queue for ~500ns and would stall compute ops
                    dmae = (nc.scalar, nc.sync, nc.gpsimd, nc.sync)[
                        (2 * m + half) % 4
                    ]
                    dmae.dma_start(
                        out=yt[m * 128 : (m + 1) * 128, c0 + half * (ntok // nq) : c0 + (half + 1) * (ntok // nq)],
                        in_=yf[:, m, hs],
                    )
            del st[i]

        # q/k/v of group i+1 are hoisted into group i's slot: they are pure PE
        # work with no softmax dependencies, so they cover the DVE-bound
        # normalize chain ahead of oproj even on the first groups (ramp).
        if n_iters > 1:
            emit_load(1)
        emit_qk(0)
        emit_v(0)
        for i in range(n_iters):
            if i + 2 < n_iters:
                emit_load(i + 2)
            if i == n_iters - 1 and i >= 1:
                # last slot has no next-group projections to cover the
                # scores->exp->attnv latency chains; interleave the previous
                # group's MLP1 chains instead
                for bi in range(gb):
                    emit_scores(i, bi)
                    emit_mlp1_chunk(i - 1, range(bi * 2, bi * 2 + 2))
                for bi in range(gb):
                    emit_attn_out(i, bi)
                    emit_mlp1_chunk(i - 1, range(8 + bi * 2, 8 + bi * 2 + 2))
                emit_outproj(i)
                emit_mlp2_store(i - 1)
                emit_mlp1_chunk(i, range(KF), last=True)
                emit_mlp2_store(i)
            else:
                for bi in range(gb):
                    emit_scores(i, bi)
                # next group's projections sit between scores and attn-outs so
                # their DVE bias-adds run ahead of the recip/normalize backlog
                # (the v chains' PSUM ring waits on those bias-adds)
                if i + 1 < n_iters:
                    emit_qk(i + 1)
                    emit_v(i + 1)
                for bi in range(gb):
                    emit_attn_out(i, bi)
                if i >= 1:
                    emit_mlp1_chunk(i - 1, range(KF))
                emit_outproj(i)
                if i >= 1:
                    emit_mlp2_store(i - 1)
        if n_iters == 1:
            emit_mlp1_chunk(0, range(KF), last=True)
            emit_mlp2_store(0)

    if mwfix:
        _fix_multiwaits(nc)
    return nc


def _get_program(ng, variant="full", repeat=1, cfg=None):
    key = ("nc", ng, variant, repeat, tuple(sorted((cfg or {}).items())))
    if key not in _cache:
        _cache[key] = _build(ng, variant, repeat, cfg)
    return _cache[key]


# --------------------------------------------------------------------------
# Host-side adaptive rounding of the fp8 MLP weights: minimize
# ||acts @ (w_opt - w)||_F over the per-element choice between the two
# neighboring fp8 grid points, by exact sequential coordinate descent
# (vectorized across output columns).
def _fp8_neighbor_toward(w, wq):
    import ml_dtypes

    b = wq.astype(ml_dtypes.float8_e4m3fn).view(np.uint8)
    sign = (b & 0x80) != 0
    mag = (b & 0x7F).astype(np.int16)
    d = w - wq
    up = d > 0
    newmag = np.where(up ^ sign, mag + 1, mag - 1)
    flip = newmag < 0
    newmag = np.where(flip, 1, newmag)
    newsign = sign ^ flip
    newmag = np.clip(newmag, 0, 0x7E)
    out = newmag.astype(np.uint8) | np.where(newsign, 0x80, 0).astype(np.uint8)
    res = out.view(ml_dtypes.float8_e4m3fn).astype(np.float32)
    return np.where(d == 0, wq, res)


def _q8(a):
    import ml_dtypes

    return np.asarray(a, np.float32).astype(ml_dtypes.float8_e4m3fn).astype(np.float32)


def _adaround(w, acts, n_sweeps=4):
    a = np.ascontiguousarray(acts, np.float32)
    G = (a.T @ a) / len(a)
    wq = _q8(w)
    alt = _fp8_neighbor_toward(w, wq)
    cur = wq.copy()
    K = w.shape[0]
    delta = cur - w
    g = G @ delta
    Gd = G.diagonal()
    for _ in range(n_sweeps):
        nflip = 0
        for i in range(K):
            other = np.where(cur[i] == wq[i], alt[i], wq[i])
            d = other - cur[i]
            gain = 2 * d * g[i] + d * d * Gd[i]
            m = gain < -1e-14
            if m.any():
                du = np.where(m, d, 0.0)
                cur[i] += du
                g += np.outer(G[:, i], du)
                nflip += int(m.sum())
        if nflip == 0:
            break
    return cur


def make_in_maps(x, wq, bq, wk, bk, wv, bv, wo, bo, w1, b1, w2, b2,
                 ada_sweeps=4, ada_sample=3072):
    import ml_dtypes

    x = np.asarray(x, np.float32)
    to_16 = lambda a: np.ascontiguousarray(np.asarray(a, np.float32).astype(MMDT_NP))

    # host-side prep: shard + transpose + cast
    ntok_total = BL * S
    x_sh = x.reshape(NCORES, ntok_total, E)
    xts = [np.ascontiguousarray(x_sh[c].T).astype(MMDT_NP) for c in range(NCORES)]

    # MLP scale convention: w1 (and b1) carry x16 so the fp8 weights sit in
    # e4m3's normal range; h tiles then hold 16*h and w2 carries x16 too, so
    # the MLP2 PSUM result is 256x and one scale of 1/256 restores it.
    # Both scalings are exact powers of two.
    w1_s = np.asarray(w1, np.float32) * 16.0
    w2_s = np.asarray(w2, np.float32) * 16.0
    b1_s = np.asarray(b1, np.float32) * 16.0

    # --- adaptive rounding of the fp8 weights, fit on a sampled-batch
    # attention forward pass so the Gram matrices see the true MLP inputs ---
    q16_ = lambda a: a.astype(np.float16).astype(np.float32)
    nbs = max(1, ada_sample // S)
    xs = x.reshape(B, S, E)[:: max(1, B // nbs)][:nbs]  # [nbs, S, E]
    xs16 = q16_(xs.reshape(nbs * S, E))
    wq16, wk16, wv16, wo16 = (
        q16_(np.asarray(w, np.float32)) for w in (wq, wk, wv, wo)
    )
    bo_eff_f = (
        np.asarray(bv, np.float64) @ np.asarray(wo, np.float64)
        + np.asarray(bo, np.float64)
    ).astype(np.float32)
    qs = q16_(xs16 @ wq16 + np.asarray(bq, np.float32))
    ks = q16_(xs16 @ wk16 + np.asarray(bk, np.float32))
    vs = xs16 @ wv16
    qh = qs.reshape(nbs, S, H, D)
    kh = ks.reshape(nbs, S, H, D)
    vh = vs.reshape(nbs, S, H, D)
    sc = np.einsum("bshd,bthd->bhst", qh, kh, optimize=True)
    sc -= sc.max(axis=-1, keepdims=True)
    p = np.exp(sc, dtype=np.float32)
    p /= p.sum(axis=-1, keepdims=True)
    attn = np.einsum("bhst,bthd->bshd", p, vh, optimize=True)
    at_s = q16_(attn.reshape(nbs * S, E))
    xm_s = q16_(q16_(at_s @ wo16 + bo_eff_f) + xs16)

    x8 = _q8(xm_s)
    axx = x8 + _q8(xm_s - x8)
    if ada_sweeps > 0:
        w1opt = _adaround(w1_s, axx, ada_sweeps)
    else:
        w1opt = _q8(w1_s)
    hprox = np.maximum(axx @ w1opt + b1_s, 0.0)
    h8 = _q8(hprox)
    ahh = h8 + _q8(hprox.astype(np.float16).astype(np.float32) - h8)
    if ada_sweeps > 0:
        w2opt = _adaround(w2_s, ahh, ada_sweeps)
    else:
        w2opt = _q8(w2_s)

    # dual-plane stationary layout [128, K, 2, M]: both planes carry the
    # same (ada-rounded) fp8 weights; the moving pair holds (q8(a), resid)
    w1pairs = np.ascontiguousarray(
        np.broadcast_to(
            w1opt.reshape(KE, 128, 1, F).transpose(1, 0, 2, 3), (128, KE, 2, F)
        )
    ).astype(ml_dtypes.float8_e4m3)
    w2pairs = np.ascontiguousarray(
        np.broadcast_to(
            w2opt.reshape(KF, 128, 1, E).transpose(1, 0, 2, 3), (128, KF, 2, E)
        )
    ).astype(ml_dtypes.float8_e4m3)

    wq_b, wk_b, wv_b, wo_b, w1_b, w2_b = map(
        to_16, (wq, wk, wv, wo, w1_s, w2_s)
    )

    resh = lambda b, nk: np.asarray(b, np.float32).reshape(nk, 128).T
    # bv is folded into the output-projection bias: P rows sum to 1, so
    # attn@wo + bo == (P@v_nobias)@wo + (bv@wo + bo).
    bo_eff = (
        np.asarray(bv, np.float64) @ np.asarray(wo, np.float64)
        + np.asarray(bo, np.float64)
    ).astype(np.float32)
    bias_pack = np.zeros((128, 32), np.float32)
    bias_pack[:, 0:KE] = resh(bq, KE)
    bias_pack[:, KE : 2 * KE] = resh(bk, KE)
    bias_pack[:, 2 * KE : 3 * KE] = resh(bo_eff, KE)
    bias_pack[:, 12 : 12 + KF] = resh(b1_s, KF)
    bias_pack[:, 28 : 28 + KE] = resh(b2, KE)

    in_maps = []
    for c in range(NCORES):
        in_maps.append(
            {
                "xt": xts[c],
                "wq": wq_b,
                "wk": wk_b,
                "wv": wv_b,
                "wo": wo_b,
                "w1p": w1pairs,
                "w2p": w2pairs,
                "w1": w1_b,
                "w2": w2_b,
                "bias": bias_pack,
            }
        )
    return in_maps


def kernel(
    x, wq, bq, wk, bk, wv, bv, wo, bo, w1, b1, w2, b2, _ng=BL // GB, _cfg=None
):
    import os

    from concourse.bass_utils import run_bass_kernel_spmd

    # The NTFF trace hook module does not exist in this container; make sure
    # run_bass_kernel_spmd never takes the trace branch even if BASS_TRACE
    # is set in the environment.
    os.environ["BASS_NEVER_TRACE"] = "1"

    in_maps = make_in_maps(x, wq, bq, wk, bk, wv, bv, wo, bo, w1, b1, w2, b2)
    ntok_total = BL * S
    cfg = dict(_cfg or {})
    # the zb fast paths fold bo_eff (= bv @ wo + bo) and b2 into fused
    # drain+residual ops; they require those biases to be exactly zero
    bo_eff = np.asarray(bv, np.float64) @ np.asarray(wo, np.float64) + np.asarray(
        bo, np.float64
    )
    if not (np.all(bo_eff == 0.0) and np.all(np.asarray(b2) == 0.0)):
        cfg["zb"] = False
    nc = _get_program(_ng, cfg=cfg)

    res = run_bass_kernel_spmd(nc, in_maps, core_ids=list(range(NCORES)))
    _cache["last_result"] = res

    out = np.empty((NCORES, ntok_total, E), np.float32)
    for c in range(NCORES):
        out[c] = res.results[c]["yt"].T.astype(np.float32)
    return out.reshape(B, S, E)
